# revision 1
# baseline (speedup 1.0000x reference)
"""Trainium2 Bass kernel for nn_ClassifierGuided (2-modality top-12-of-16 MoE classifier).

Sharding: pure data-parallel over tokens. 2 modalities x 4096 tokens = 8192
tokens; each of the 8 cores owns 1024 tokens of one modality (cores 0-3 ->
modality 0, cores 4-7 -> modality 1) and that modality's full weights.
Dense-eval MoE (all 16 experts computed, sparse gates applied), so no
all-to-all is needed.

Per-core math (transposed activation layout, d on partitions):
  gating   : logits = x @ Wg -> top-12 mask -> softmax -> gates g [B,16]
  experts  : h_e = relu(x @ W1_e + b1_e); hg_e = g_e * h_e
  combine  : moeT = sum_e W2_e^T @ hg_e  (+ b2^T @ g^T folded as one matmul)
  residual : z = relu(moe) + x
  head     : outT = Wo^T @ zT + bo

All matmuls run as float32r (full-rate fp32 PE path, ~1e-4 relative rounding).
Experts are processed in pairs so every expert matmul is a full 128x128 tile
(192+192 = 384 = 3*128 h-dims per pair).
"""
import sys

sys.path.insert(0, "/opt/trn_rl_repo")

import numpy as np

import concourse.bass as bass
import concourse.mybir as mybir
import concourse.tile as tile
from concourse import bacc
from concourse.bass_utils import run_bass_kernel_spmd
from concourse.masks import make_identity

# ---- problem sizes (hardcoded per the harness contract) ----
B = 4096           # tokens per modality
D = 768            # model dim
E = 16             # experts
H = 192            # expert hidden
O = 101            # classifier out
KTOP = 12          # top-k experts
NCORES = 8
BC = B // 4        # 1024 tokens per core
DC = D // 128      # 6 d-chunks
NT = 512           # token tile (matmul moving dim)
NTILES = BC // NT  # 2
NPAIR = E // 2     # 8 expert pairs
HP = 2 * H         # 384 h-dims per pair = 3 chunks of 128
HCH = HP // 128    # 3
F32 = mybir.dt.float32
F32R = mybir.dt.float32r
NEG_BIG = -1.0e30

_NC_CACHE = {}
DEBUG = False


def build_nc():
    nc = bacc.Bacc("TRN2", target_bir_lowering=False, debug=False,
                   num_devices=NCORES)

    # ---- DRAM I/O (per-core views; host pre-packs) ----
    xT = nc.dram_tensor("xT", [D, BC], F32R, kind="ExternalInput").ap()
    w1p = nc.dram_tensor("w1p", [D, E * H], F32R, kind="ExternalInput").ap()
    w2p = nc.dram_tensor("w2p", [E * H, D], F32R, kind="ExternalInput").ap()
    b1p = nc.dram_tensor("b1p", [128, E * H // 128], F32, kind="ExternalInput").ap()
    b2 = nc.dram_tensor("b2", [E, D], F32R, kind="ExternalInput").ap()
    wg = nc.dram_tensor("wg", [D, E], F32R, kind="ExternalInput").ap()
    wo = nc.dram_tensor("wo", [D, O], F32R, kind="ExternalInput").ap()
    bo = nc.dram_tensor("bo", [O, 1], F32, kind="ExternalInput").ap()
    outT = nc.dram_tensor("outT", [O, BC], F32, kind="ExternalOutput").ap()
    if DEBUG:
        dbg_gT = nc.dram_tensor("dbg_gT", [E, BC], F32, kind="ExternalOutput").ap()
        dbg_gb = nc.dram_tensor("dbg_gb", [128, 2, NT], F32, kind="ExternalOutput").ap()
        dbg_hg = nc.dram_tensor("dbg_hg", [128, NT], F32, kind="ExternalOutput").ap()
        dbg_h = nc.dram_tensor("dbg_h", [128, NT], F32, kind="ExternalOutput").ap()
        dbg_moe = nc.dram_tensor("dbg_moe", [128, DC, NT], F32, kind="ExternalOutput").ap()

    with tile.TileContext(nc) as tc:
        with tc.tile_pool(name="const", bufs=1) as cpool:
            # resident SBUF tensors
            xsb = cpool.tile([128, DC, BC], F32R)          # xT; later holds zT
            w1sb = cpool.tile([128, DC, E * H], F32R)
            b1sb = cpool.tile([128, E * H // 128], F32)
            b2sb = cpool.tile([E, D], F32R)
            wosb = cpool.tile([128, DC, O], F32R)
            bosb = cpool.tile([O, 1], F32)
            gT = cpool.tile([E, BC], F32R)                 # gates, expert-major
            wgf = cpool.tile([128, DC, E], F32)            # fp32 gating weights
            ident = cpool.tile([128, 128], F32)

            make_identity(nc, ident[:, :])

            # loads ordered by first use: wg + x (gating) split across the SP
            # and ACT HWDGE queues, then W1 by pair interleaved with the small
            # tensors so pair 0 lands as early as possible
            nc.sync.dma_start(out=wgf[:, :, :],
                              in_=wg.bitcast(F32).rearrange("(c p) e -> p c e", p=128))
            for c in range(DC):
                eng = nc.sync if c < 3 else nc.scalar
                eng.dma_start(out=xsb[:, c, :], in_=xT[128 * c:128 * (c + 1), :])
            w1v = w1p.rearrange("(c q) h -> q c h", q=128)

            def load_w1(p):
                nc.sync.dma_start(out=w1sb[:, :, HP * p:HP * (p + 1)],
                                  in_=w1v[:, :, HP * p:HP * (p + 1)])

            w2tiles = {}
            ctx_w2 = tc.tile_pool(name="w2pool", bufs=3)
            w2pool = ctx_w2.__enter__()

            def load_w2(t, p):
                # W2 on the SP queue (not ACT: transfers there block the
                # latency-critical relu chain); one DMA per pair
                w2 = w2pool.tile([128, HCH, D], F32R, tag="w2", name="w2t")
                nc.sync.dma_start(
                    out=w2[:, :, :],
                    in_=w2p[HP * p:HP * (p + 1), :].rearrange(
                        "(m q) d -> q m d", q=128))
                w2tiles[(t, p)] = w2

            load_w1(0)
            nc.sync.dma_start(out=b1sb[:, :], in_=b1p[:, :])
            load_w1(1)
            load_w2(0, 0)
            load_w1(2)
            load_w2(0, 1)
            nc.sync.dma_start(out=b2sb[:, :], in_=b2[:, :])
            load_w1(3)
            load_w2(0, 2)
            for c in range(DC):
                nc.sync.dma_start(out=wosb[:, c, :], in_=wo[128 * c:128 * (c + 1), :])
            nc.sync.dma_start(out=bosb[:, :], in_=bo[:, :])
            for p in range(4, NPAIR):
                load_w1(p)
                load_w2(0, p - 1)
            load_w2(0, NPAIR - 1)

            # gates round-trip through DRAM; gate-broadcast tiles are read
            # back with a partition-step-0 DMA (POOL partition_broadcast is
            # HW-limited to out-base-partition 0)
            gdram = cpool.tile([E, BC], F32R, space="DRAM")
            gdram_ap = gdram
            gb_pre = {}
            gbpool = ctx_gb = tc.tile_pool(name="gbpool", bufs=2)
            gbpool = ctx_gb.__enter__()

            def load_gb(t, p):
                # gb[:,0,:] = gate(e0) broadcast over partitions; [:,1,:] = e1
                gb = gbpool.tile([128, 2, NT], F32R, tag="gb", name="gb")
                gb_src = bass.AP(tensor=gdram.tensor,
                                 offset=2 * p * BC + NT * t,
                                 ap=[[0, 128], [BC, 2], [1, NT]])
                nc.gpsimd.dma_start(out=gb[:, :, :], in_=gb_src)
                return gb

            # ---------------- gating pass (128-token subtiles) ----------------
            with tc.tile_pool(name="gps", bufs=2, space="PSUM") as gps, \
                 tc.tile_pool(name="gtp", bufs=2, space="PSUM") as gtp, \
                 tc.tile_pool(name="gsb", bufs=3) as gsb, \
                 tc.tile_pool(name="xgpool", bufs=2) as xgpool:
                xTv32 = xT.bitcast(F32).rearrange("(c q) b -> q c b", q=128)
                for i in range(BC // 128):
                    if i * 128 % NT == 0 and i > 0:
                        # flush finished token-half of the gates to DRAM early
                        lo = i * 128 - NT
                        nc.gpsimd.dma_start(out=gdram_ap[:, lo:lo + NT],
                                            in_=gT[:, lo:lo + NT])
                        if lo == 0:
                            gb_pre[(0, 0)] = load_gb(0, 0)
                            gb_pre[(0, 1)] = load_gb(0, 1)
                    ts = slice(128 * i, 128 * (i + 1))
                    # fp32-typed copies so the logits matmul runs in exact fp32
                    # (top-12 selection then matches the fp32 reference)
                    xg = xgpool.tile([128, DC, 128], F32, tag="xg", name="xg")
                    nc.gpsimd.dma_start(out=xg[:, :, :], in_=xTv32[:, :, ts])
                    lg_ps = gps.tile([128, E], F32, tag="lg")
                    for c in range(DC):
                        nc.tensor.matmul(lg_ps[:, :], xg[:, c, :], wgf[:, c, :],
                                         start=(c == 0), stop=(c == DC - 1))
                    lg = gsb.tile([128, E], F32, tag="lg_sb")
                    nc.vector.tensor_copy(lg[:, :], lg_ps[:, :])
                    # top-8 values, then values 9..16 after masking them out
                    t8a = gsb.tile([128, 8], F32, tag="t8a")
                    nc.vector.max(t8a[:, :], lg[:, :])
                    l2 = gsb.tile([128, E], F32, tag="l2")
                    nc.vector.match_replace(l2[:, :], t8a[:, :], lg[:, :], NEG_BIG)
                    t8b = gsb.tile([128, 8], F32, tag="t8b")
                    nc.vector.max(t8b[:, :], l2[:, :])
                    # softmax over entries >= 12th-largest (t8b[:,3])
                    e16 = gsb.tile([128, E], F32, tag="e16")
                    nc.scalar.activation(e16[:, :], lg[:, :],
                                         mybir.ActivationFunctionType.Exp)
                    em = gsb.tile([128, E], F32, tag="em")
                    ssum = gsb.tile([128, 1], F32, tag="ssum")
                    nc.vector.scalar_tensor_tensor(
                        out=em[:, :], in0=lg[:, :], scalar=t8b[:, 3:4],
                        in1=e16[:, :], op0=mybir.AluOpType.is_ge,
                        op1=mybir.AluOpType.mult, accum_out=ssum[:, :])
                    rinv = gsb.tile([128, 1], F32, tag="rinv")
                    nc.vector.reciprocal(rinv[:, :], ssum[:, :])
                    g = gsb.tile([128, E], F32, tag="g")
                    nc.vector.tensor_scalar_mul(g[:, :], em[:, :], rinv[:, :])
                    # transpose to expert-major gT[16, tokens]
                    gt_ps = gtp.tile([E, 128], F32, tag="gt")
                    nc.tensor.transpose(gt_ps[:, :], g[:, :], ident[:, :])
                    nc.vector.tensor_copy(gT[:, ts], gt_ps[:, :])
            if DEBUG:
                nc.sync.dma_start(out=dbg_gT[:, :], in_=gT[:, :].bitcast(F32))

            nc.gpsimd.dma_start(out=gdram_ap[:, BC - NT:], in_=gT[:, BC - NT:])

            # ---------------- main loop ----------------
            with tc.tile_pool(name="moeps", bufs=DC, space="PSUM") as moeps, \
                 tc.tile_pool(name="hps", bufs=2, space="PSUM") as hps, \
                 tc.tile_pool(name="gstpool", bufs=2) as gstpool, \
                 tc.tile_pool(name="hgpool", bufs=(20 if DEBUG else 22)) as hgpool, \
                 tc.tile_pool(name="opool", bufs=2) as opool:
                for t in range(NTILES):
                    ts = slice(NT * t, NT * (t + 1))
                    # one PSUM tile per d-chunk: a single big tile would put a
                    # false tile-level WAR between chunk c's drain (DVE read)
                    # and chunk c+1's accumulation (PE write)
                    moe = [moeps.tile([128, NT], F32, tag="moe", name="moe")
                           for _ in range(DC)]
                    w2t = [None] * NPAIR
                    hg = [[None] * HCH for _ in range(NPAIR)]

                    def stage1(p, ts=ts, moe=moe, w2t=w2t, hg=hg, t=t):
                        w2t[p] = w2tiles.pop((t, p), None)
                        if w2t[p] is None:
                            load_w2(t, p)
                            w2t[p] = w2tiles.pop((t, p))
                        gb = gb_pre.pop((t, p), None)
                        if gb is None:
                            gb = load_gb(t, p)
                        if DEBUG and t == 0 and p == 0:
                            nc.sync.dma_start(out=dbg_gb[:, :, :], in_=gb[:, :, :].bitcast(F32))
                        for m in range(HCH):
                            hcol = HP * p + 128 * m
                            hps_t = hps.tile([128, NT], F32, tag="h")
                            for c in range(DC):
                                nc.tensor.matmul(hps_t[:, :],
                                                 w1sb[:, c, hcol:hcol + 128],
                                                 xsb[:, c, ts],
                                                 start=(c == 0), stop=(c == DC - 1))
                            # relu(u + b1) in-place in PSUM, then gate-multiply to SBUF
                            nc.scalar.activation(hps_t[:, :], hps_t[:, :],
                                                 mybir.ActivationFunctionType.Relu,
                                                 bias=b1sb[:, hcol // 128:hcol // 128 + 1])
                            if DEBUG and t == 0 and p == 0 and m == 0:
                                dbg_h_sb = gstpool.tile([128, NT], F32, tag="dbg", name="dbg_h_sb")
                                nc.vector.tensor_copy(dbg_h_sb[:, :], hps_t[:, :])
                                nc.sync.dma_start(out=dbg_h[:, :], in_=dbg_h_sb[:, :])
                            hg[p][m] = hgpool.tile([128, NT], F32R, tag="hg", name="hg")
                            if m == 1:
                                # mixed chunk: parts 0:64 are e0's h[128:192],
                                # parts 64:128 are e1's h[0:64]
                                nc.vector.tensor_tensor(
                                    out=hg[p][m][0:64, :], in0=hps_t[0:64, :],
                                    in1=gb[0:64, 0, :].bitcast(F32),
                                    op=mybir.AluOpType.mult)
                                nc.vector.tensor_tensor(
                                    out=hg[p][m][64:128, :], in0=hps_t[64:128, :],
                                    in1=gb[64:128, 1, :].bitcast(F32),
                                    op=mybir.AluOpType.mult)
                            else:
                                nc.vector.tensor_tensor(
                                    out=hg[p][m][:, :], in0=hps_t[:, :],
                                    in1=gb[:, 0 if m == 0 else 1, :].bitcast(F32),
                                    op=mybir.AluOpType.mult)
                            if DEBUG and t == 0 and p == 0 and m == 0:
                                nc.sync.dma_start(out=dbg_hg[:, :], in_=hg[p][m][:, :].bitcast(F32))

                    def stage2(p, moe=moe, w2t=w2t, hg=hg, ts=ts, close=False):
                        if not close:
                            # m-outer: the first 12 matmuls need only hg m0/m1,
                            # giving hg m2's relu+mult chain extra cover
                            for m in range(HCH):
                                for c in range(DC):
                                    nc.tensor.matmul(moe[c][:, :],
                                                     w2t[p][:, m, 128 * c:128 * (c + 1)],
                                                     hg[p][m][:, :],
                                                     start=(p == 0 and m == 0), stop=False)
                            return
                        for c in range(DC):
                            for m in range(HCH):
                                nc.tensor.matmul(moe[c][:, :],
                                                 w2t[p][:, m, 128 * c:128 * (c + 1)],
                                                 hg[p][m][:, :],
                                                 start=(p == 0 and m == 0), stop=False)
                            if close:
                                # b2 bias term closes this chunk's accumulation
                                nc.tensor.matmul(moe[c][:, :],
                                                 b2sb[:, 128 * c:128 * (c + 1)],
                                                 gT[:, ts], start=False, stop=True)
                                finish_chunk(c)
                                # head matmul trails two chunks behind so its
                                # relu+residual drain is already complete
                                if c >= 2:
                                    head_chunk(c - 2)
                        if close:
                            head_chunk(DC - 2)
                            head_chunk(DC - 1)

                    def finish_chunk(c, moe=moe, ts=ts):
                        # z = relu(moe) + x in one DVE op, overwriting x in place
                        if DEBUG and t == 0:
                            dbg_moe_sb = gstpool.tile([128, NT], F32, tag="dbg", name="dbg_moe_sb")
                            nc.vector.tensor_copy(dbg_moe_sb[:, :], moe[c][:, :])
                            nc.sync.dma_start(out=dbg_moe[:, c, :], in_=dbg_moe_sb[:, :])
                        nc.vector.scalar_tensor_tensor(
                            out=xsb[:, c, ts], in0=moe[c][:, :], scalar=0.0,
                            in1=xsb[:, c, ts].bitcast(F32),
                            op0=mybir.AluOpType.max, op1=mybir.AluOpType.add)

                    out_ps_box = [None]

                    def head_chunk(c, ts=ts):
                        if out_ps_box[0] is None:
                            out_ps_box[0] = hps.tile([O, NT], F32, tag="h",
                                                     name="out_ps")
                        nc.tensor.matmul(out_ps_box[0][:, :], wosb[:, c, :],
                                         xsb[:, c, ts],
                                         start=(c == 0), stop=(c == DC - 1))

                    # software pipeline: stage1(p+1) covers stage2(p) latency;
                    # the last pair closes each moe chunk so relu/residual/head
                    # drain per chunk while later chunks still accumulate
                    stage1(0)
                    for p in range(NPAIR):
                        if p + 1 < NPAIR:
                            stage1(p + 1)
                        stage2(p, close=(p == NPAIR - 1))
                    out_ps = out_ps_box[0]
                    osb = opool.tile([O, NT], F32, tag="osb")
                    nc.scalar.activation(osb[:, :], out_ps[:, :],
                                         mybir.ActivationFunctionType.Identity,
                                         bias=bosb[:, :])
                    nc.sync.dma_start(out=outT[:, ts], in_=osb[:, :])
            ctx_gb.__exit__(None, None, None)
            ctx_w2.__exit__(None, None, None)

    nc.compile()
    return nc


def _pack_core_inputs(x, Wg, W1, b1, W2, b2, Wo, bo, c4):
    """Per-core input dict for one modality's weights + 1024-token slice."""
    f = np.float32
    tok = slice(BC * c4, BC * (c4 + 1))
    return {
        "xT": np.ascontiguousarray(np.asarray(x[tok], f).T),
        "w1p": np.ascontiguousarray(np.asarray(W1, f).transpose(1, 0, 2).reshape(D, E * H)),
        "w2p": np.ascontiguousarray(np.asarray(W2, f).reshape(E * H, D)),
        "b1p": np.ascontiguousarray(np.asarray(b1, f).reshape(-1).reshape(E * H // 128, 128).T),
        "b2": np.ascontiguousarray(np.asarray(b2, f)),
        "wg": np.ascontiguousarray(np.asarray(Wg, f)),
        "wo": np.ascontiguousarray(np.asarray(Wo, f)),
        "bo": np.ascontiguousarray(np.asarray(bo, f).reshape(O, 1)),
    }


def run_on_hw(inputs, trace=False, **kw):
    if "nc" not in _NC_CACHE:
        _NC_CACHE["nc"] = build_nc()
    nc = _NC_CACHE["nc"]
    in_maps = []
    for core in range(NCORES):
        i, c4 = divmod(core, 4)
        x = inputs["x0"] if i == 0 else inputs["x1"]
        in_maps.append(_pack_core_inputs(
            x, inputs["Wg"][i], inputs["W1"][i], inputs["b1"][i],
            inputs["W2"][i], inputs["b2"][i], inputs["Wo"][i], inputs["bo"][i], c4))
    res = run_bass_kernel_spmd(nc, in_maps, core_ids=list(range(NCORES)),
                               trace=trace, **kw)
    outs = []
    for i in range(2):
        outs.append(np.concatenate(
            [res.results[4 * i + c]["outT"].T for c in range(4)], axis=0))
    return (outs[0], outs[1]), res


def kernel(**inputs):
    (o0, o1), _ = run_on_hw(inputs)
    return (o0, o1)



# revision 8
# speedup vs baseline: 1.9021x; 1.9021x over previous
"""Trainium2 Bass kernel for nn_ClassifierGuided (2-modality top-12-of-16 MoE classifier).

Sharding: pure data-parallel over tokens. 2 modalities x 4096 tokens; each of
the 8 cores owns 1024 tokens of one modality (cores 0-3 -> modality 0, cores
4-7 -> modality 1) and that modality's full weights. Dense-eval MoE (all 16
experts computed, sparse gates applied), so no all-to-all is needed.

Expert matmuls run in fp8 (e4m3) with the PE DoubleRow perf mode: each matmul
contracts 256 k-dims (2 packed rows per partition) at 0.5 cycles per output
row -- 4x the fp32r rate. Scale scheme keeps every fused op exact:
  xsb   = 512*x   (fp32, gating/residual/head path; Wg and Wo divided by 512)
  x8    = x       (fp8 moving operand of W1)
  w18   = 64*W1, w28 = 64*W2, gates cast to fp8 as 8*g, b28 = 512*b2
  h_psum = 64*(x@W1) -> ACT relu(in/64 + b1) -> h (true scale, fp32 in PSUM)
  hg8   = h * (8g) in fp8 (DVE/Pool, fused multiply + cast)
  moe_psum = hg8@(64*W2) + (8g)@(512*b2 / 8)... = 512*(moe+b2term)
  drain: z' = max(moe_psum,0) + xsb = 512*z (single DVE op), head uses Wo/512.

Gating is exact fp32: (512x)@(Wg/512) is bit-identical to x@Wg, so top-12
selection matches the reference.
"""
import sys

sys.path.insert(0, "/opt/trn_rl_repo")

import ml_dtypes
import numpy as np

import concourse.bass as bass
import concourse.mybir as mybir
import concourse.tile as tile
from concourse import bacc
from concourse.bass_utils import run_bass_kernel_spmd
from concourse.masks import make_identity

# ---- problem sizes (hardcoded per the harness contract) ----
B = 4096           # tokens per modality
D = 768            # model dim
E = 16             # experts
H = 192            # expert hidden
O = 101            # classifier out
KTOP = 12          # top-k experts
NCORES = 8
BC = B // 4        # 1024 tokens per core
DC = D // 128      # 6 d-chunks
NT = 512           # token tile (matmul moving dim)
NTILES = BC // NT  # 2
NHC = E * H // 128 # 24 h-chunks of 128
NK1 = D // 256     # 3 DoubleRow k-tiles for W1 (contract d)
NK2 = E * H // 256 # 12 DoubleRow k-tiles for W2 (contract h)
NPAIR = E // 2     # 8 expert pairs (one gb broadcast tile each)
HALF_W1 = E * H // 2
F32 = mybir.dt.float32
F32R = mybir.dt.float32r
F8 = mybir.dt.float8e4
DR = mybir.MatmulPerfMode.DoubleRow
NEG_BIG = -1.0e30

SX = 512.0   # x residual scale (Wg, Wo pre-divided on host)
SW = 64.0    # fp8 weight scale for W1/W2
SG = 8.0     # fp8 gate scale  (SW * SG == SX keeps the drain exact)

_NC_CACHE = {}


def build_nc():
    nc = bacc.Bacc("TRN2", target_bir_lowering=False, debug=False,
                   num_devices=NCORES)

    # ---- DRAM I/O (per-core views; host pre-packs) ----
    xT = nc.dram_tensor("xT", [D, BC], F32R, kind="ExternalInput").ap()       # 512*x
    x8d = nc.dram_tensor("x8d", [128, 2, NK1, BC], F8, kind="ExternalInput").ap()
    w18d = nc.dram_tensor("w18d", [128, 2, NK1, E * H], F8, kind="ExternalInput").ap()
    w28d = nc.dram_tensor("w28d", [128, 2, NK2, D], F8, kind="ExternalInput").ap()
    b1p = nc.dram_tensor("b1p", [128, NHC], F32, kind="ExternalInput").ap()
    b28d = nc.dram_tensor("b28d", [8, 2, D], F8, kind="ExternalInput").ap()   # 512*b2/8
    wg = nc.dram_tensor("wg", [D, E], F32, kind="ExternalInput").ap()         # Wg/512
    wo = nc.dram_tensor("wo", [D, O], F32R, kind="ExternalInput").ap()        # Wo/512
    bo = nc.dram_tensor("bo", [O, 1], F32, kind="ExternalInput").ap()
    outT = nc.dram_tensor("outT", [O, BC], F32, kind="ExternalOutput").ap()

    with tile.TileContext(nc) as tc:
        with tc.tile_pool(name="const", bufs=1) as cpool:
            # resident SBUF tensors
            xsb = cpool.tile([128, DC, BC], F32R)            # 512*xT; later 512*zT
            x8 = cpool.tile([128, 2, NK1, BC], F8)
            w18 = cpool.tile([128, 2, NK1, E * H], F8)
            w28 = cpool.tile([128, 2, NK2, D], F8)
            b1sb = cpool.tile([128, NHC], F32)
            b28 = cpool.tile([8, 2, D], F8)
            wgf = cpool.tile([128, DC, E], F32)
            wosb = cpool.tile([128, DC, O], F32R)
            bosb = cpool.tile([O, 1], F32)
            gT = cpool.tile([E, BC], F32)                    # gates, expert-major
            gT8 = cpool.tile([E, BC], F8)                    # 8*gates, fp8
            gTa8 = cpool.tile([8, 2, BC], F8)                # 8*gates, DoubleRow view
            ident = cpool.tile([128, 128], F32)

            make_identity(nc, ident[:, :])

            # ---- loads, per-queue FIFO in need order ----
            # SP: gating weights + first x chunks (gating starts ASAP), then
            # fp8 x; gate flush/broadcast DMAs are emitted inline later.
            # ACT: remaining x chunks (ACT compute starts only at first relu).
            # Pool: fp8 expert weights (w18 first; w28 k-tiles in use order).
            nc.sync.dma_start(out=wgf[:, :, :],
                              in_=wg.rearrange("(c p) e -> p c e", p=128))
            for c in range(DC):
                eng = nc.sync if c < 3 else nc.scalar
                eng.dma_start(out=xsb[:, c, :], in_=xT[128 * c:128 * (c + 1), :])
            nc.sync.dma_start(out=b1sb[:, :], in_=b1p[:, :])
            for kt in range(NK1):
                nc.gpsimd.dma_start(out=w18[:, :, kt, 0:HALF_W1],
                                    in_=w18d[:, :, kt, 0:HALF_W1])
                nc.sync.dma_start(out=x8[:, :, kt, :], in_=x8d[:, :, kt, :])
            for kt in range(NK1):
                nc.gpsimd.dma_start(out=w18[:, :, kt, HALF_W1:],
                                    in_=w18d[:, :, kt, HALF_W1:])
            for g2 in range(NK2 // 2):
                nc.gpsimd.dma_start(out=w28[:, :, 2 * g2:2 * g2 + 2, :],
                                    in_=w28d[:, :, 2 * g2:2 * g2 + 2, :])
            nc.gpsimd.dma_start(out=b28[:, :, :], in_=b28d)
            for c in range(DC):
                nc.gpsimd.dma_start(out=wosb[:, c, :],
                                    in_=wo[128 * c:128 * (c + 1), :])
            nc.gpsimd.dma_start(out=bosb[:, :], in_=bo[:, :])

            # gates round-trip through DRAM (fp8); gate-broadcast tiles are
            # read back with a partition-step-0 DMA
            gdram8 = cpool.tile([E, BC], F8, space="DRAM")
            gb_tiles = {}
            ctx_gb = tc.tile_pool(name="gbpool", bufs=2 * NPAIR)
            gbpool = ctx_gb.__enter__()

            def load_gb(t, p):
                # gb[:,0,:] = 8*gate(e0) broadcast over partitions; [:,1,:] = e1
                gb = gbpool.tile([128, 2, NT], F8, tag="gb", name="gb")
                gb_src = bass.AP(tensor=gdram8.tensor,
                                 offset=2 * p * BC + NT * t,
                                 ap=[[0, 128], [BC, 2], [1, NT]])
                nc.sync.dma_start(out=gb[:, :, :], in_=gb_src)
                gb_tiles[(t, p)] = gb

            def flush_gates(t):
                ts = slice(NT * t, NT * (t + 1))
                # cast fp32 gates -> 8*g in fp8, then flush for broadcast reads
                nc.vector.tensor_scalar_mul(gT8[:, ts], gT[:, ts], SG)
                nc.sync.dma_start(out=gdram8[:, ts], in_=gT8[:, ts])
                # DoubleRow-packed gate view for the b2 bias matmul
                ga_src = bass.AP(tensor=gdram8.tensor, offset=NT * t,
                                 ap=[[BC, 8], [8 * BC, 2], [1, NT]])
                nc.sync.dma_start(out=gTa8[:, :, ts], in_=ga_src)
                for p in range(NPAIR):
                    load_gb(t, p)

            # ---------------- gating pass (128-token subtiles) ----------------
            with tc.tile_pool(name="gps", bufs=2, space="PSUM") as gps, \
                 tc.tile_pool(name="gtp", bufs=2, space="PSUM") as gtp, \
                 tc.tile_pool(name="gsb", bufs=3) as gsb:
                for i in range(BC // 128):
                    if i * 128 % NT == 0 and i > 0:
                        flush_gates(i * 128 // NT - 1)
                    ts = slice(128 * i, 128 * (i + 1))
                    # (512x)@(Wg/512) in exact fp32: bit-identical to x@Wg, so
                    # the top-12 selection matches the fp32 reference
                    lg_ps = gps.tile([128, E], F32, tag="lg")
                    for c in range(DC):
                        nc.tensor.matmul(lg_ps[:, :], xsb[:, c, ts].bitcast(F32),
                                         wgf[:, c, :],
                                         start=(c == 0), stop=(c == DC - 1))
                    lg = gsb.tile([128, E], F32, tag="lg_sb")
                    nc.vector.tensor_copy(lg[:, :], lg_ps[:, :])
                    # top-8 values, then values 9..16 after masking them out
                    t8a = gsb.tile([128, 8], F32, tag="t8a")
                    nc.vector.max(t8a[:, :], lg[:, :])
                    l2 = gsb.tile([128, E], F32, tag="l2")
                    nc.vector.match_replace(l2[:, :], t8a[:, :], lg[:, :], NEG_BIG)
                    t8b = gsb.tile([128, 8], F32, tag="t8b")
                    nc.vector.max(t8b[:, :], l2[:, :])
                    # softmax over entries >= 12th-largest (t8b[:,3])
                    e16 = gsb.tile([128, E], F32, tag="e16")
                    nc.scalar.activation(e16[:, :], lg[:, :],
                                         mybir.ActivationFunctionType.Exp)
                    em = gsb.tile([128, E], F32, tag="em")
                    ssum = gsb.tile([128, 1], F32, tag="ssum")
                    nc.vector.scalar_tensor_tensor(
                        out=em[:, :], in0=lg[:, :], scalar=t8b[:, 3:4],
                        in1=e16[:, :], op0=mybir.AluOpType.is_ge,
                        op1=mybir.AluOpType.mult, accum_out=ssum[:, :])
                    rinv = gsb.tile([128, 1], F32, tag="rinv")
                    nc.vector.reciprocal(rinv[:, :], ssum[:, :])
                    g = gsb.tile([128, E], F32, tag="g")
                    nc.vector.tensor_scalar_mul(g[:, :], em[:, :], rinv[:, :])
                    # transpose to expert-major gT[16, tokens]
                    gt_ps = gtp.tile([E, 128], F32, tag="gt")
                    nc.tensor.transpose(gt_ps[:, :], g[:, :], ident[:, :])
                    nc.vector.tensor_copy(gT[:, ts], gt_ps[:, :])

            flush_gates(NTILES - 1)

            # ---------------- main loop ----------------
            # per tile: 24 W1 h-chunks (3 DoubleRow matmuls each), relu on ACT,
            # gate-mult+fp8-cast on DVE/Pool, then 12 W2 k-tiles (6 DoubleRow
            # matmuls each) accumulating into 6 moe PSUM banks; the last k-tile
            # closes with the b2 bias matmul, the relu/residual drain, and the
            # trailing head matmuls.
            with tc.tile_pool(name="moeps", bufs=DC, space="PSUM") as moeps, \
                 tc.tile_pool(name="hps", bufs=2, space="PSUM") as hps, \
                 tc.tile_pool(name="hgpool", bufs=2) as hgpool, \
                 tc.tile_pool(name="opool", bufs=2) as opool:
                for t in range(NTILES):
                    ts = slice(NT * t, NT * (t + 1))
                    moe = [moeps.tile([128, NT], F32, tag="moe", name="moe")
                           for _ in range(DC)]
                    hg8 = hgpool.tile([128, 2, NK2, NT], F8, tag="hg", name="hg")

                    def stage1_chunk(hc, ts=ts, hg8=hg8, t=t):
                        # W1 + relu + gate-mult for h-chunk hc (h rows 128hc..)
                        p, m = divmod(hc, 3)
                        hps_t = hps.tile([128, NT], F32, tag="h")
                        for kt in range(NK1):
                            nc.tensor.matmul(hps_t[:, :],
                                             w18[:, :, kt, 128 * hc:128 * (hc + 1)],
                                             x8[:, :, kt, ts],
                                             start=(kt == 0), stop=(kt == NK1 - 1),
                                             perf_mode=DR)
                        # h = relu(h_psum/64 + b1), in place in PSUM
                        nc.scalar.activation(hps_t[:, :], hps_t[:, :],
                                             mybir.ActivationFunctionType.Relu,
                                             bias=b1sb[:, hc:hc + 1], scale=1.0 / SW)
                        # hg8 = h * (8g), fused multiply + fp8 cast. During
                        # tile 0 the Pool queue is still draining weight DMAs,
                        # so tile 0 runs all mults on DVE; tile 1 alternates.
                        eng = nc.vector if (t == 0 or hc % 2 == 0) else nc.gpsimd
                        gb = gb_tiles[(t, p)]
                        dst = hg8[:, hc % 2, hc // 2, :]
                        if m == 1:
                            # mixed chunk: parts 0:64 are e0's h[128:192],
                            # parts 64:128 are e1's h[0:64]
                            eng.tensor_tensor(out=dst[0:64, :], in0=hps_t[0:64, :],
                                              in1=gb[0:64, 0, :],
                                              op=mybir.AluOpType.mult)
                            eng.tensor_tensor(out=dst[64:128, :], in0=hps_t[64:128, :],
                                              in1=gb[64:128, 1, :],
                                              op=mybir.AluOpType.mult)
                        else:
                            eng.tensor_tensor(out=dst[:, :], in0=hps_t[:, :],
                                              in1=gb[:, 0 if m == 0 else 1, :],
                                              op=mybir.AluOpType.mult)

                    out_ps_box = [None]

                    def head_chunk(c, ts=ts):
                        if out_ps_box[0] is None:
                            out_ps_box[0] = hps.tile([O, NT], F32, tag="h",
                                                     name="out_ps")
                        nc.tensor.matmul(out_ps_box[0][:, :], wosb[:, c, :],
                                         xsb[:, c, ts],
                                         start=(c == 0), stop=(c == DC - 1))

                    def finish_chunk(c, moe=moe, ts=ts):
                        # z' = max(moe_psum,0) + 512x in one DVE op, in place
                        nc.vector.scalar_tensor_tensor(
                            out=xsb[:, c, ts], in0=moe[c][:, :], scalar=0.0,
                            in1=xsb[:, c, ts].bitcast(F32),
                            op0=mybir.AluOpType.max, op1=mybir.AluOpType.add)

                    def stage2_ktile(k, close, moe=moe, hg8=hg8, ts=ts):
                        for c in range(DC):
                            nc.tensor.matmul(moe[c][:, :],
                                             w28[:, :, k, 128 * c:128 * (c + 1)],
                                             hg8[:, :, k, :],
                                             start=(k == 0), stop=False,
                                             perf_mode=DR)
                            if close:
                                # b2 bias term closes this chunk's accumulation:
                                # (8g) @ (512*b2/8) adds 512*(g@b2)
                                nc.tensor.matmul(moe[c][:, :],
                                                 b28[:, :, 128 * c:128 * (c + 1)],
                                                 gTa8[:, :, ts],
                                                 start=False, stop=True,
                                                 perf_mode=DR)
                                finish_chunk(c)
                                # head matmul trails two chunks behind so its
                                # relu+residual drain is already complete
                                if c >= 2:
                                    head_chunk(c - 2)
                        if close:
                            head_chunk(DC - 2)
                            head_chunk(DC - 1)

                    # software pipeline: W1 chunks run 2-3 ahead of the W2
                    # k-tile that consumes them, covering the relu+mult chain
                    for k in range(NK2):
                        if k == 0:
                            for hc in range(4):
                                stage1_chunk(hc)
                        else:
                            for hc in (2 * k + 2, 2 * k + 3):
                                if hc < NHC:
                                    stage1_chunk(hc)
                        stage2_ktile(k, close=(k == NK2 - 1))

                    out_ps = out_ps_box[0]
                    osb = opool.tile([O, NT], F32, tag="osb")
                    nc.scalar.activation(osb[:, :], out_ps[:, :],
                                         mybir.ActivationFunctionType.Identity,
                                         bias=bosb[:, :])
                    nc.sync.dma_start(out=outT[:, ts], in_=osb[:, :])
            ctx_gb.__exit__(None, None, None)

    nc.compile()
    return nc


def _pack_core_inputs(x, Wg, W1, b1, W2, b2, Wo, bo, c4):
    """Per-core input dict for one modality's weights + 1024-token slice."""
    f = np.float32
    f8 = ml_dtypes.float8_e4m3
    tok = slice(BC * c4, BC * (c4 + 1))
    x = np.asarray(x[tok], f)                                    # [BC, 768]
    xT = np.ascontiguousarray(x.T)                               # [768, BC]
    # DoubleRow pack: k = 256*kt + 128*i + p  ->  [p, i, kt, ...]
    x8 = np.ascontiguousarray(
        xT.reshape(NK1, 2, 128, BC).transpose(2, 1, 0, 3)).astype(f8)
    W1r = np.asarray(W1, f).transpose(1, 0, 2).reshape(D, E * H)  # [768, 3072]
    w18 = np.ascontiguousarray(
        (SW * W1r).reshape(NK1, 2, 128, E * H).transpose(2, 1, 0, 3)).astype(f8)
    W2r = np.asarray(W2, f).reshape(E * H, D)                     # [3072, 768]
    w28 = np.ascontiguousarray(
        (SW * W2r).reshape(NK2, 2, 128, D).transpose(2, 1, 0, 3)).astype(f8)
    # b2 DoubleRow pack: expert e = p + 8*i; scaled so (8g)@b28 = 512*(g@b2)
    b28 = np.ascontiguousarray(
        (SX / SG) * np.asarray(b2, f).reshape(2, 8, D).transpose(1, 0, 2)).astype(f8)
    return {
        "xT": np.ascontiguousarray(SX * xT),
        "x8d": x8,
        "w18d": w18,
        "w28d": w28,
        "b1p": np.ascontiguousarray(np.asarray(b1, f).reshape(NHC, 128).T),
        "b28d": b28,
        "wg": np.ascontiguousarray(np.asarray(Wg, f) / SX),
        "wo": np.ascontiguousarray(np.asarray(Wo, f) / SX),
        "bo": np.ascontiguousarray(np.asarray(bo, f).reshape(O, 1)),
    }


def run_on_hw(inputs, trace=False, **kw):
    if "nc" not in _NC_CACHE:
        _NC_CACHE["nc"] = build_nc()
    nc = _NC_CACHE["nc"]
    in_maps = []
    for core in range(NCORES):
        i, c4 = divmod(core, 4)
        x = inputs["x0"] if i == 0 else inputs["x1"]
        in_maps.append(_pack_core_inputs(
            x, inputs["Wg"][i], inputs["W1"][i], inputs["b1"][i],
            inputs["W2"][i], inputs["b2"][i], inputs["Wo"][i], inputs["bo"][i], c4))
    res = run_bass_kernel_spmd(nc, in_maps, core_ids=list(range(NCORES)),
                               trace=trace, **kw)
    outs = []
    for i in range(2):
        outs.append(np.concatenate(
            [res.results[4 * i + c]["outT"].T for c in range(4)], axis=0))
    return (outs[0], outs[1]), res


def kernel(**inputs):
    (o0, o1), _ = run_on_hw(inputs)
    return (o0, o1)


# revision 13
# speedup vs baseline: 2.0989x; 1.1035x over previous
"""Trainium2 Bass kernel for nn_ClassifierGuided (2-modality top-12-of-16 MoE classifier).

Sharding: pure data-parallel over tokens. 2 modalities x 4096 tokens; each of
the 8 cores owns 1024 tokens of one modality (cores 0-3 -> modality 0, cores
4-7 -> modality 1) and that modality's full weights. Dense-eval MoE (all 16
experts computed, sparse gates applied), so no all-to-all is needed.

Expert matmuls run in fp8 (e4m3) with the PE DoubleRow perf mode: each matmul
contracts 256 k-dims (2 packed rows per partition) at 0.5 cycles per output
row -- 4x the fp32r rate. Scale scheme keeps every fused op exact:
  xsb   = 512*x   (fp32, gating/residual/head path; Wg and Wo divided by 512)
  x8    = x       (fp8 moving operand of W1)
  w18   = 64*W1, w28 = 64*W2, gates cast to fp8 as 8*g, b28 = 512*b2
  h_psum = 64*(x@W1) -> ACT relu(in/64 + b1) -> h (true scale, fp32 in PSUM)
  hg8   = h * (8g) in fp8 (DVE/Pool, fused multiply + cast)
  moe_psum = hg8@(64*W2) + (8g)@(512*b2 / 8)... = 512*(moe+b2term)
  drain: z' = max(moe_psum,0) + xsb = 512*z (single DVE op), head uses Wo/512.

Gating is exact fp32: (512x)@(Wg/512) is bit-identical to x@Wg, so top-12
selection matches the reference.
"""
import sys

sys.path.insert(0, "/opt/trn_rl_repo")

import ml_dtypes
import numpy as np

import concourse.bass as bass
import concourse.mybir as mybir
import concourse.tile as tile
from concourse import bacc
from concourse.bass_utils import run_bass_kernel_spmd
from concourse.masks import make_identity

# ---- problem sizes (hardcoded per the harness contract) ----
B = 4096           # tokens per modality
D = 768            # model dim
E = 16             # experts
H = 192            # expert hidden
O = 101            # classifier out
KTOP = 12          # top-k experts
NCORES = 8
BC = B // 4        # 1024 tokens per core
DC = D // 128      # 6 d-chunks
NT = 512           # token tile (matmul moving dim)
NTILES = BC // NT  # 2
NHC = E * H // 128 # 24 h-chunks of 128
NK1 = D // 256     # 3 DoubleRow k-tiles for W1 (contract d)
NK2 = E * H // 256 # 12 DoubleRow k-tiles for W2 (contract h)
NPAIR = E // 2     # 8 expert pairs (one gb broadcast tile each)
HALF_W1 = E * H // 2
F32 = mybir.dt.float32
F32R = mybir.dt.float32r
F8 = mybir.dt.float8e4
DR = mybir.MatmulPerfMode.DoubleRow
NEG_BIG = -1.0e30

SX = 512.0   # x residual scale (Wg, Wo pre-divided on host)
SW = 64.0    # fp8 weight scale for W1/W2
SG = 8.0     # fp8 gate scale  (SW * SG == SX keeps the drain exact)

_NC_CACHE = {}


def build_nc():
    nc = bacc.Bacc("TRN2", target_bir_lowering=False, debug=False,
                   num_devices=NCORES)

    # ---- DRAM I/O (per-core views; host pre-packs) ----
    xT = nc.dram_tensor("xT", [D, BC], F32R, kind="ExternalInput").ap()       # 512*x
    x8d = nc.dram_tensor("x8d", [128, 2, NK1, BC], F8, kind="ExternalInput").ap()
    w18d = nc.dram_tensor("w18d", [128, 2, NK1, E * H], F8, kind="ExternalInput").ap()
    w28d = nc.dram_tensor("w28d", [128, 2, NK2, D], F8, kind="ExternalInput").ap()
    b1p = nc.dram_tensor("b1p", [128, NHC], F32, kind="ExternalInput").ap()
    b28d = nc.dram_tensor("b28d", [8, 2, D], F8, kind="ExternalInput").ap()   # 512*b2/8
    wg = nc.dram_tensor("wg", [D, E], F32, kind="ExternalInput").ap()         # Wg/512
    wo = nc.dram_tensor("wo", [D, O], F32R, kind="ExternalInput").ap()        # Wo/512
    bo = nc.dram_tensor("bo", [O, 1], F32, kind="ExternalInput").ap()
    outT = nc.dram_tensor("outT", [O, BC], F32, kind="ExternalOutput").ap()

    with tile.TileContext(nc) as tc:
        with tc.tile_pool(name="const", bufs=1) as cpool:
            # resident SBUF tensors
            xsb = cpool.tile([128, DC, BC], F32R)            # 512*xT; later 512*zT
            x8 = cpool.tile([128, 2, NK1, BC], F8)
            w18 = cpool.tile([128, 2, NK1, E * H], F8)
            w28 = cpool.tile([128, 2, NK2, D], F8)
            b1sb = cpool.tile([128, NHC], F32)
            b28 = cpool.tile([8, 2, D], F8)
            wgf = cpool.tile([128, DC, E], F32)
            wosb = cpool.tile([128, DC, O], F32R)
            bosb = cpool.tile([O, 1], F32)
            gT = cpool.tile([E, BC], F32)                    # gates, expert-major
            gT8 = cpool.tile([E, BC], F8)                    # 8*gates, fp8
            gTa8 = cpool.tile([8, 2, BC], F8)                # 8*gates, DoubleRow view
            ident = cpool.tile([128, 128], F32)

            make_identity(nc, ident[:, :])

            # ---- loads, per-queue FIFO in need order ----
            # SP: gating weights + first x chunks (gating starts ASAP), then
            # fp8 x; gate flush/broadcast DMAs are emitted inline later.
            # ACT: remaining x chunks (ACT compute starts only at first relu).
            # Pool: fp8 expert weights (w18 first; w28 k-tiles in use order).
            nc.sync.dma_start(out=wgf[:, :, :],
                              in_=wg.rearrange("(c p) e -> p c e", p=128))
            for c in range(DC):
                eng = nc.sync if c % 2 == 0 else nc.scalar
                eng.dma_start(out=xsb[:, c, :], in_=xT[128 * c:128 * (c + 1), :])
            nc.sync.dma_start(out=b1sb[:, :], in_=b1p[:, :])
            for kt in range(NK1):
                nc.gpsimd.dma_start(out=w18[:, :, kt, 0:HALF_W1],
                                    in_=w18d[:, :, kt, 0:HALF_W1])
                nc.sync.dma_start(out=x8[:, :, kt, :], in_=x8d[:, :, kt, :])
            for kt in range(NK1):
                nc.gpsimd.dma_start(out=w18[:, :, kt, HALF_W1:],
                                    in_=w18d[:, :, kt, HALF_W1:])
            for g2 in range(NK2 // 2):
                nc.gpsimd.dma_start(out=w28[:, :, 2 * g2:2 * g2 + 2, :],
                                    in_=w28d[:, :, 2 * g2:2 * g2 + 2, :])
            nc.gpsimd.dma_start(out=b28[:, :, :], in_=b28d)
            for c in range(DC):
                nc.gpsimd.dma_start(out=wosb[:, c, :],
                                    in_=wo[128 * c:128 * (c + 1), :])
            nc.gpsimd.dma_start(out=bosb[:, :], in_=bo[:, :])

            # gates round-trip through DRAM (fp8); gate-broadcast tiles are
            # read back with a partition-step-0 DMA
            gdram8 = cpool.tile([E, BC], F8, space="DRAM")
            gb_tiles = {}
            gbmix = {}
            ctx_gb = tc.tile_pool(name="gbpool", bufs=2 * NPAIR + 2)
            gbpool = ctx_gb.__enter__()

            def load_gb(t, p):
                # gb[:,0,:] = 8*gate(e0) broadcast over partitions; [:,1,:] = e1
                gb = gbpool.tile([128, 2, NT], F8, tag="gb", name="gb")
                gb_src = bass.AP(tensor=gdram8.tensor,
                                 offset=2 * p * BC + NT * t,
                                 ap=[[0, 128], [BC, 2], [1, NT]])
                nc.sync.dma_start(out=gb[:, :, :], in_=gb_src)
                gb_tiles[(t, p)] = gb

            def load_gbmix(t):
                # mixed-chunk gate tile: for pair p, partitions 0:64 carry
                # e(2p) and 64:128 carry e(2p+1) -- matches the partition
                # split of the middle h-chunk, so its gate-mult is one op
                gm = gbpool.tile([128, NPAIR, NT], F8, tag="gbm", name="gbm")
                for half in range(2):
                    src = bass.AP(tensor=gdram8.tensor,
                                  offset=half * BC + NT * t,
                                  ap=[[0, 64], [2 * BC, NPAIR], [1, NT]])
                    nc.sync.dma_start(out=gm[64 * half:64 * (half + 1), :, :],
                                      in_=src)
                gbmix[t] = gm

            def flush_gates(t):
                ts = slice(NT * t, NT * (t + 1))
                # cast fp32 gates -> 8*g in fp8, then flush for broadcast reads
                nc.vector.tensor_scalar_mul(gT8[:, ts], gT[:, ts], SG)
                nc.sync.dma_start(out=gdram8[:, ts], in_=gT8[:, ts])
                # DoubleRow-packed gate view for the b2 bias matmul
                ga_src = bass.AP(tensor=gdram8.tensor, offset=NT * t,
                                 ap=[[BC, 8], [8 * BC, 2], [1, NT]])
                nc.sync.dma_start(out=gTa8[:, :, ts], in_=ga_src)
                load_gb(t, 0)
                load_gbmix(t)
                for p in range(1, NPAIR):
                    load_gb(t, p)

            # ---------------- gating pass (128-token subtiles) ----------------
            with tc.tile_pool(name="gps", bufs=2, space="PSUM") as gps, \
                 tc.tile_pool(name="gtp", bufs=2, space="PSUM") as gtp, \
                 tc.tile_pool(name="gsb", bufs=3) as gsb:
                for i in range(BC // 128):
                    if i * 128 % NT == 0 and i > 0:
                        flush_gates(i * 128 // NT - 1)
                    ts = slice(128 * i, 128 * (i + 1))
                    # (512x)@(Wg/512) in exact fp32: bit-identical to x@Wg, so
                    # the top-12 selection matches the fp32 reference
                    lg_ps = gps.tile([128, E], F32, tag="lg")
                    for c in range(DC):
                        nc.tensor.matmul(lg_ps[:, :], xsb[:, c, ts].bitcast(F32),
                                         wgf[:, c, :],
                                         start=(c == 0), stop=(c == DC - 1))
                    lg = gsb.tile([128, E], F32, tag="lg_sb")
                    nc.vector.tensor_copy(lg[:, :], lg_ps[:, :])
                    # top-8 values, then values 9..16 after masking them out
                    t8a = gsb.tile([128, 8], F32, tag="t8a")
                    nc.vector.max(t8a[:, :], lg[:, :])
                    l2 = gsb.tile([128, E], F32, tag="l2")
                    nc.vector.match_replace(l2[:, :], t8a[:, :], lg[:, :], NEG_BIG)
                    t8b = gsb.tile([128, 8], F32, tag="t8b")
                    nc.vector.max(t8b[:, :], l2[:, :])
                    # softmax over entries >= 12th-largest (t8b[:,3])
                    e16 = gsb.tile([128, E], F32, tag="e16")
                    nc.scalar.activation(e16[:, :], lg[:, :],
                                         mybir.ActivationFunctionType.Exp)
                    em = gsb.tile([128, E], F32, tag="em")
                    ssum = gsb.tile([128, 1], F32, tag="ssum")
                    nc.vector.scalar_tensor_tensor(
                        out=em[:, :], in0=lg[:, :], scalar=t8b[:, 3:4],
                        in1=e16[:, :], op0=mybir.AluOpType.is_ge,
                        op1=mybir.AluOpType.mult, accum_out=ssum[:, :])
                    rinv = gsb.tile([128, 1], F32, tag="rinv")
                    nc.vector.reciprocal(rinv[:, :], ssum[:, :])
                    g = gsb.tile([128, E], F32, tag="g")
                    nc.vector.tensor_scalar_mul(g[:, :], em[:, :], rinv[:, :])
                    # transpose to expert-major gT[16, tokens]
                    gt_ps = gtp.tile([E, 128], F32, tag="gt")
                    nc.tensor.transpose(gt_ps[:, :], g[:, :], ident[:, :])
                    nc.vector.tensor_copy(gT[:, ts], gt_ps[:, :])

            flush_gates(NTILES - 1)

            # ---------------- main loop ----------------
            # per tile: 24 W1 h-chunks (3 DoubleRow matmuls each), relu on ACT,
            # gate-mult+fp8-cast on DVE/Pool, then 12 W2 k-tiles (6 DoubleRow
            # matmuls each) accumulating into 6 moe PSUM banks; the last k-tile
            # closes with the b2 bias matmul, the relu/residual drain, and the
            # trailing head matmuls.
            with tc.tile_pool(name="moeps", bufs=DC, space="PSUM") as moeps, \
                 tc.tile_pool(name="hps", bufs=2, space="PSUM") as hps, \
                 tc.tile_pool(name="hgpool", bufs=2) as hgpool, \
                 tc.tile_pool(name="opool", bufs=2) as opool:
                for t in range(NTILES):
                    ts = slice(NT * t, NT * (t + 1))
                    moe = [moeps.tile([128, NT], F32, tag="moe", name="moe")
                           for _ in range(DC)]
                    hg8 = hgpool.tile([128, 2, NK2, NT], F8, tag="hg", name="hg")

                    def stage1_chunk(hc, ts=ts, hg8=hg8, t=t):
                        # W1 + relu + gate-mult for h-chunk hc (h rows 128hc..)
                        p, m = divmod(hc, 3)
                        hps_t = hps.tile([128, NT], F32, tag="h")
                        for kt in range(NK1):
                            nc.tensor.matmul(hps_t[:, :],
                                             w18[:, :, kt, 128 * hc:128 * (hc + 1)],
                                             x8[:, :, kt, ts],
                                             start=(kt == 0), stop=(kt == NK1 - 1),
                                             perf_mode=DR)
                        # h = relu(h_psum/64 + b1), in place in PSUM
                        nc.scalar.activation(hps_t[:, :], hps_t[:, :],
                                             mybir.ActivationFunctionType.Relu,
                                             bias=b1sb[:, hc:hc + 1], scale=1.0 / SW)
                        # hg8 = h * (8g), fused multiply + fp8 cast. During
                        # tile 0 the Pool queue is still draining weight DMAs,
                        # so tile 0 runs all mults on DVE; tile 1 mostly Pool.
                        eng = nc.vector if (t == 0 or m == 0) else nc.gpsimd
                        dst = hg8[:, hc % 2, hc // 2, :]
                        if m == 1:
                            gbs = gbmix[t][:, p, :]
                        else:
                            gbs = gb_tiles[(t, p)][:, 0 if m == 0 else 1, :]
                        eng.tensor_tensor(out=dst[:, :], in0=hps_t[:, :],
                                          in1=gbs,
                                          op=mybir.AluOpType.mult)

                    out_ps_box = [None]

                    def head_chunk(c, ts=ts):
                        if out_ps_box[0] is None:
                            out_ps_box[0] = hps.tile([O, NT], F32, tag="h",
                                                     name="out_ps")
                        nc.tensor.matmul(out_ps_box[0][:, :], wosb[:, c, :],
                                         xsb[:, c, ts],
                                         start=(c == 0), stop=(c == DC - 1))

                    def finish_chunk(c, moe=moe, ts=ts):
                        # z' = max(moe_psum,0) + 512x in one Pool op, in place
                        # (Pool's weight DMAs have drained by the first close)
                        nc.gpsimd.scalar_tensor_tensor(
                            out=xsb[:, c, ts], in0=moe[c][:, :], scalar=0.0,
                            in1=xsb[:, c, ts].bitcast(F32),
                            op0=mybir.AluOpType.max, op1=mybir.AluOpType.add)

                    def stage2_ktile(k, close, moe=moe, hg8=hg8, ts=ts):
                        for c in range(DC):
                            nc.tensor.matmul(moe[c][:, :],
                                             w28[:, :, k, 128 * c:128 * (c + 1)],
                                             hg8[:, :, k, :],
                                             start=(k == 0), stop=False,
                                             perf_mode=DR)
                            if close:
                                # b2 bias term closes this chunk's accumulation:
                                # (8g) @ (512*b2/8) adds 512*(g@b2)
                                nc.tensor.matmul(moe[c][:, :],
                                                 b28[:, :, 128 * c:128 * (c + 1)],
                                                 gTa8[:, :, ts],
                                                 start=False, stop=True,
                                                 perf_mode=DR)
                                finish_chunk(c)
                                # head matmul trails two chunks behind so its
                                # relu+residual drain is already complete
                                if c >= 2:
                                    head_chunk(c - 2)
                        if close:
                            head_chunk(DC - 2)
                            head_chunk(DC - 1)

                    # software pipeline: W1 chunks run 2-3 ahead of the W2
                    # k-tile that consumes them, covering the relu+mult chain
                    for k in range(NK2):
                        if k == 0:
                            for hc in range(4):
                                stage1_chunk(hc)
                        else:
                            for hc in (2 * k + 2, 2 * k + 3):
                                if hc < NHC:
                                    stage1_chunk(hc)
                        stage2_ktile(k, close=(k == NK2 - 1))

                    out_ps = out_ps_box[0]
                    osb = opool.tile([O, NT], F32, tag="osb")
                    for hf in range(2):
                        hs = slice(NT // 2 * hf, NT // 2 * (hf + 1))
                        nc.scalar.activation(osb[:, hs], out_ps[:, hs],
                                             mybir.ActivationFunctionType.Identity,
                                             bias=bosb[:, :])
                        nc.sync.dma_start(out=outT[:, NT * t + NT // 2 * hf:
                                                   NT * t + NT // 2 * (hf + 1)],
                                          in_=osb[:, hs])
            ctx_gb.__exit__(None, None, None)

    nc.compile()
    return nc


def _pack_core_inputs(x, Wg, W1, b1, W2, b2, Wo, bo, c4):
    """Per-core input dict for one modality's weights + 1024-token slice."""
    f = np.float32
    f8 = ml_dtypes.float8_e4m3
    tok = slice(BC * c4, BC * (c4 + 1))
    x = np.asarray(x[tok], f)                                    # [BC, 768]
    xT = np.ascontiguousarray(x.T)                               # [768, BC]
    # DoubleRow pack: k = 256*kt + 128*i + p  ->  [p, i, kt, ...]
    x8 = np.ascontiguousarray(
        xT.reshape(NK1, 2, 128, BC).transpose(2, 1, 0, 3)).astype(f8)
    W1r = np.asarray(W1, f).transpose(1, 0, 2).reshape(D, E * H)  # [768, 3072]
    w18 = np.ascontiguousarray(
        (SW * W1r).reshape(NK1, 2, 128, E * H).transpose(2, 1, 0, 3)).astype(f8)
    W2r = np.asarray(W2, f).reshape(E * H, D)                     # [3072, 768]
    w28 = np.ascontiguousarray(
        (SW * W2r).reshape(NK2, 2, 128, D).transpose(2, 1, 0, 3)).astype(f8)
    # b2 DoubleRow pack: expert e = p + 8*i; scaled so (8g)@b28 = 512*(g@b2)
    b28 = np.ascontiguousarray(
        (SX / SG) * np.asarray(b2, f).reshape(2, 8, D).transpose(1, 0, 2)).astype(f8)
    return {
        "xT": np.ascontiguousarray(SX * xT),
        "x8d": x8,
        "w18d": w18,
        "w28d": w28,
        "b1p": np.ascontiguousarray(np.asarray(b1, f).reshape(NHC, 128).T),
        "b28d": b28,
        "wg": np.ascontiguousarray(np.asarray(Wg, f) / SX),
        "wo": np.ascontiguousarray(np.asarray(Wo, f) / SX),
        "bo": np.ascontiguousarray(np.asarray(bo, f).reshape(O, 1)),
    }


def run_on_hw(inputs, trace=False, **kw):
    if "nc" not in _NC_CACHE:
        _NC_CACHE["nc"] = build_nc()
    nc = _NC_CACHE["nc"]
    in_maps = []
    for core in range(NCORES):
        i, c4 = divmod(core, 4)
        x = inputs["x0"] if i == 0 else inputs["x1"]
        in_maps.append(_pack_core_inputs(
            x, inputs["Wg"][i], inputs["W1"][i], inputs["b1"][i],
            inputs["W2"][i], inputs["b2"][i], inputs["Wo"][i], inputs["bo"][i], c4))
    res = run_bass_kernel_spmd(nc, in_maps, core_ids=list(range(NCORES)),
                               trace=trace, **kw)
    outs = []
    for i in range(2):
        outs.append(np.concatenate(
            [res.results[4 * i + c]["outT"].T for c in range(4)], axis=0))
    return (outs[0], outs[1]), res


def kernel(**inputs):
    (o0, o1), _ = run_on_hw(inputs)
    return (o0, o1)


# revision 14
# speedup vs baseline: 2.5398x; 1.2101x over previous
"""Trainium2 Bass kernel for nn_ClassifierGuided (2-modality top-12-of-16 MoE classifier).

Sharding: pure data-parallel over tokens. 2 modalities x 4096 tokens; each of
the 8 cores owns 1024 tokens of one modality (cores 0-3 -> modality 0, cores
4-7 -> modality 1) and that modality's full weights. Dense-eval MoE (all 16
experts computed, sparse gates applied), so no all-to-all is needed.

Expert matmuls run in fp8 (e4m3) with the PE DoubleRow perf mode: each matmul
contracts 256 k-dims (2 packed rows per partition) at 0.5 cycles per output
row -- 4x the fp32r rate. Scale scheme keeps every fused op cheap:
  xsb   = 512*x   (bf16; gating/residual/head path; Wg, Wo divided by 512)
  x8    = x       (fp8 moving operand of W1)
  w18   = 64*W1, w28 = 64*W2, gates cast to fp8 as 8*g, b28 = 512*b2/8
  h_psum = 64*(x@W1) -> ACT relu(in/64 + b1) -> h parked in SBUF (fp32)
  hg8   = h * (8g) in fp8 (DVE/Pool, fused multiply + cast)
  moe_psum = hg8@(64*W2) + (8g)@(512*b2/8) = 512*(moe + b2 term)
  drain: z' = max(moe_psum,0) + xsb = 512*z (single fused op), head uses Wo/512.

Parking h in SBUF frees each h-PSUM bank right after the relu, so the
gate-broadcast DMA latency never blocks the W1 pipeline.
"""
import sys

sys.path.insert(0, "/opt/trn_rl_repo")

import ml_dtypes
import numpy as np

import concourse.bass as bass
import concourse.mybir as mybir
import concourse.tile as tile
from concourse import bacc
from concourse.bass_utils import run_bass_kernel_spmd
from concourse.masks import make_identity

# ---- problem sizes (hardcoded per the harness contract) ----
B = 4096           # tokens per modality
D = 768            # model dim
E = 16             # experts
H = 192            # expert hidden
O = 101            # classifier out
KTOP = 12          # top-k experts
NCORES = 8
BC = B // 4        # 1024 tokens per core
DC = D // 128      # 6 d-chunks
NT = 512           # token tile (matmul moving dim)
NTILES = BC // NT  # 2
NHC = E * H // 128 # 24 h-chunks of 128
NK1 = D // 256     # 3 DoubleRow k-tiles for W1 (contract d)
NK2 = E * H // 256 # 12 DoubleRow k-tiles for W2 (contract h)
NPAIR = E // 2     # 8 expert pairs (one gb broadcast tile each)
F32 = mybir.dt.float32
BF16 = mybir.dt.bfloat16
F8 = mybir.dt.float8e4
DR = mybir.MatmulPerfMode.DoubleRow
NEG_BIG = -1.0e30

SX = 512.0   # x residual scale (Wg, Wo pre-divided on host)
SW = 64.0    # fp8 weight scale for W1/W2
SG = 8.0     # fp8 gate scale  (SW * SG == SX keeps the drain exact)

_NC_CACHE = {}


def build_nc():
    nc = bacc.Bacc("TRN2", target_bir_lowering=False, debug=False,
                   num_devices=NCORES)

    # ---- DRAM I/O (per-core views; host pre-packs) ----
    xT = nc.dram_tensor("xT", [D, BC], BF16, kind="ExternalInput").ap()       # 512*x
    x8d = nc.dram_tensor("x8d", [128, 2, NK1, BC], F8, kind="ExternalInput").ap()
    w18d = nc.dram_tensor("w18d", [128, 2, NK1, E * H], F8, kind="ExternalInput").ap()
    w28d = nc.dram_tensor("w28d", [128, 2, NK2, D], F8, kind="ExternalInput").ap()
    b1p = nc.dram_tensor("b1p", [128, NHC], F32, kind="ExternalInput").ap()
    b28d = nc.dram_tensor("b28d", [8, 2, D], F8, kind="ExternalInput").ap()
    wg = nc.dram_tensor("wg", [D, E], BF16, kind="ExternalInput").ap()        # Wg/512
    wo = nc.dram_tensor("wo", [D, O], BF16, kind="ExternalInput").ap()        # Wo/512
    bo = nc.dram_tensor("bo", [O, 1], F32, kind="ExternalInput").ap()
    outT = nc.dram_tensor("outT", [O, BC], F32, kind="ExternalOutput").ap()

    with tile.TileContext(nc) as tc:
        with tc.tile_pool(name="const", bufs=1) as cpool:
            # resident SBUF tensors
            xsb = cpool.tile([128, DC, BC], BF16)            # 512*xT; later 512*zT
            x8 = cpool.tile([128, 2, NK1, BC], F8)
            w18 = cpool.tile([128, 2, NK1, E * H], F8)
            w28 = cpool.tile([128, 2, NK2, D], F8)
            b1sb = cpool.tile([128, NHC], F32)
            b28 = cpool.tile([8, 2, D], F8)
            wgf = cpool.tile([128, DC, E], BF16)
            wosb = cpool.tile([128, DC, O], BF16)
            bosb = cpool.tile([O, 1], F32)
            gT = cpool.tile([E, BC], F32)                    # gates, expert-major
            gT8 = cpool.tile([E, BC], F8)                    # 8*gates, fp8
            gTa8 = cpool.tile([8, 2, BC], F8)                # 8*gates, DoubleRow view
            ident = cpool.tile([128, 128], F32)

            make_identity(nc, ident[:, :])

            # ---- loads, per-queue FIFO in need order ----
            # SP: gating weights + half of x (gating-critical), then the gate
            # flush/broadcast DMAs (emitted inline) and the output writes.
            # ACT: other half of x (ACT compute starts only at first relu).
            # Pool: fp8 x + expert weights, interleaved in use order.
            nc.sync.dma_start(out=wgf[:, :, :],
                              in_=wg.rearrange("(c p) e -> p c e", p=128))
            for c in (0, 2, 4):
                nc.sync.dma_start(out=xsb[:, c, :], in_=xT[128 * c:128 * (c + 1), :])
            nc.sync.dma_start(out=b1sb[:, :], in_=b1p[:, :])
            for c in (1, 3, 5):
                nc.scalar.dma_start(out=xsb[:, c, :], in_=xT[128 * c:128 * (c + 1), :])
            # Pool, in use order: x8 + first W1 columns, then W2 k-tiles
            # interleaved with later W1 columns so both streams stay ahead.
            for kt in range(NK1):
                nc.gpsimd.dma_start(out=x8[:, :, kt, :], in_=x8d[:, :, kt, :])
            HQ = E * H // 4
            for kt in range(NK1):
                nc.gpsimd.dma_start(out=w18[:, :, kt, 0:HQ],
                                    in_=w18d[:, :, kt, 0:HQ])
            for g2 in range(3):
                nc.gpsimd.dma_start(out=w28[:, :, 2 * g2:2 * g2 + 2, :],
                                    in_=w28d[:, :, 2 * g2:2 * g2 + 2, :])
                for kt in range(NK1):
                    nc.gpsimd.dma_start(
                        out=w18[:, :, kt, HQ * (g2 + 1):HQ * (g2 + 2)],
                        in_=w18d[:, :, kt, HQ * (g2 + 1):HQ * (g2 + 2)])
            for g2 in range(3, 6):
                nc.gpsimd.dma_start(out=w28[:, :, 2 * g2:2 * g2 + 2, :],
                                    in_=w28d[:, :, 2 * g2:2 * g2 + 2, :])
            nc.gpsimd.dma_start(out=b28[:, :, :], in_=b28d)
            for c in range(DC):
                nc.gpsimd.dma_start(out=wosb[:, c, :],
                                    in_=wo[128 * c:128 * (c + 1), :])
            nc.gpsimd.dma_start(out=bosb[:, :], in_=bo[:, :])

            # gates round-trip through DRAM (fp8); gate-broadcast tiles are
            # read back with partition-step-0 DMAs
            gdram8 = cpool.tile([E, BC], F8, space="DRAM")
            gb_tiles = {}
            gbmix = {}
            ctx_gb = tc.tile_pool(name="gbpool", bufs=2 * NPAIR + 2)
            gbpool = ctx_gb.__enter__()

            def load_gb(t, p):
                # gb[:,0,:] = 8*gate(e0) broadcast over partitions; [:,1,:] = e1
                gb = gbpool.tile([128, 2, NT], F8, tag="gb", name="gb")
                gb_src = bass.AP(tensor=gdram8.tensor,
                                 offset=2 * p * BC + NT * t,
                                 ap=[[0, 128], [BC, 2], [1, NT]])
                nc.sync.dma_start(out=gb[:, :, :], in_=gb_src)
                gb_tiles[(t, p)] = gb

            def load_gbmix(t):
                # mixed-chunk gate tile: for pair p, partitions 0:64 carry
                # e(2p) and 64:128 carry e(2p+1) -- matches the partition
                # split of the middle h-chunk, so its gate-mult is one op
                gm = gbpool.tile([128, NPAIR, NT], F8, tag="gbm", name="gbm")
                for half in range(2):
                    src = bass.AP(tensor=gdram8.tensor,
                                  offset=half * BC + NT * t,
                                  ap=[[0, 64], [2 * BC, NPAIR], [1, NT]])
                    nc.sync.dma_start(out=gm[64 * half:64 * (half + 1), :, :],
                                      in_=src)
                gbmix[t] = gm

            def flush_gates(t):
                ts = slice(NT * t, NT * (t + 1))
                # cast fp32 gates -> 8*g in fp8, then flush for broadcast reads
                nc.vector.tensor_scalar_mul(gT8[:, ts], gT[:, ts], SG)
                nc.sync.dma_start(out=gdram8[:, ts], in_=gT8[:, ts])
                # DoubleRow-packed gate view for the b2 bias matmul
                ga_src = bass.AP(tensor=gdram8.tensor, offset=NT * t,
                                 ap=[[BC, 8], [8 * BC, 2], [1, NT]])
                nc.sync.dma_start(out=gTa8[:, :, ts], in_=ga_src)
                load_gb(t, 0)
                load_gbmix(t)
                for p in range(1, NPAIR):
                    load_gb(t, p)

            # ---------------- main-loop machinery ----------------
            ctx_hps = tc.tile_pool(name="hps", bufs=2, space="PSUM")
            hps = ctx_hps.__enter__()
            ctx_hpark = tc.tile_pool(name="hpark", bufs=6)
            hpark_pool = ctx_hpark.__enter__()
            hpark = {}

            def stage1a(t, hc):
                # W1 (fp8 DoubleRow) + relu; h parked in SBUF, PSUM bank freed
                ts = slice(NT * t, NT * (t + 1))
                hps_t = hps.tile([128, NT], F32, tag="h")
                for kt in range(NK1):
                    nc.tensor.matmul(hps_t[:, :],
                                     w18[:, :, kt, 128 * hc:128 * (hc + 1)],
                                     x8[:, :, kt, ts],
                                     start=(kt == 0), stop=(kt == NK1 - 1),
                                     perf_mode=DR)
                hp = hpark_pool.tile([128, NT], F32, tag="hp", name="hp")
                nc.scalar.activation(hp[:, :], hps_t[:, :],
                                     mybir.ActivationFunctionType.Relu,
                                     bias=b1sb[:, hc:hc + 1], scale=1.0 / SW)
                hpark[(t, hc)] = hp

            # ---------------- gating pass (128-token subtiles) ----------------
            # W1 for the first few h-chunks is emitted first: it only needs
            # x8/w18, so the PE warms up while the bf16 x chunks still stream.
            for hc in range(4):
                stage1a(0, hc)

            with tc.tile_pool(name="gps", bufs=2, space="PSUM") as gps, \
                 tc.tile_pool(name="gtp", bufs=2, space="PSUM") as gtp, \
                 tc.tile_pool(name="gsb", bufs=3) as gsb:
                for i in range(BC // 128):
                    if i * 128 % NT == 0 and i > 0:
                        flush_gates(i * 128 // NT - 1)
                    ts = slice(128 * i, 128 * (i + 1))
                    lg_ps = gps.tile([128, E], F32, tag="lg")
                    for c in range(DC):
                        nc.tensor.matmul(lg_ps[:, :], xsb[:, c, ts],
                                         wgf[:, c, :],
                                         start=(c == 0), stop=(c == DC - 1))
                    lg = gsb.tile([128, E], F32, tag="lg_sb")
                    nc.vector.tensor_copy(lg[:, :], lg_ps[:, :])
                    # top-8 values, then values 9..16 after masking them out
                    t8a = gsb.tile([128, 8], F32, tag="t8a")
                    nc.vector.max(t8a[:, :], lg[:, :])
                    l2 = gsb.tile([128, E], F32, tag="l2")
                    nc.vector.match_replace(l2[:, :], t8a[:, :], lg[:, :], NEG_BIG)
                    t8b = gsb.tile([128, 8], F32, tag="t8b")
                    nc.vector.max(t8b[:, :], l2[:, :])
                    # softmax over entries >= 12th-largest (t8b[:,3])
                    e16 = gsb.tile([128, E], F32, tag="e16")
                    nc.scalar.activation(e16[:, :], lg[:, :],
                                         mybir.ActivationFunctionType.Exp)
                    em = gsb.tile([128, E], F32, tag="em")
                    ssum = gsb.tile([128, 1], F32, tag="ssum")
                    nc.vector.scalar_tensor_tensor(
                        out=em[:, :], in0=lg[:, :], scalar=t8b[:, 3:4],
                        in1=e16[:, :], op0=mybir.AluOpType.is_ge,
                        op1=mybir.AluOpType.mult, accum_out=ssum[:, :])
                    rinv = gsb.tile([128, 1], F32, tag="rinv")
                    nc.vector.reciprocal(rinv[:, :], ssum[:, :])
                    g = gsb.tile([128, E], F32, tag="g")
                    nc.vector.tensor_scalar_mul(g[:, :], em[:, :], rinv[:, :])
                    # transpose to expert-major gT[16, tokens]
                    gt_ps = gtp.tile([E, 128], F32, tag="gt")
                    nc.tensor.transpose(gt_ps[:, :], g[:, :], ident[:, :])
                    nc.vector.tensor_copy(gT[:, ts], gt_ps[:, :])

            flush_gates(NTILES - 1)

            # ---------------- main loop ----------------
            with tc.tile_pool(name="moeps", bufs=DC, space="PSUM") as moeps, \
                 tc.tile_pool(name="hgpool", bufs=2) as hgpool, \
                 tc.tile_pool(name="opool", bufs=2) as opool:
                for t in range(NTILES):
                    ts = slice(NT * t, NT * (t + 1))
                    moe = [moeps.tile([128, NT], F32, tag="moe", name="moe")
                           for _ in range(DC)]
                    hg8 = hgpool.tile([128, 2, NK2, NT], F8, tag="hg", name="hg")

                    def mult_chunk(hc, t=t, hg8=hg8):
                        # hg8 = h * (8g): fused multiply + fp8 cast from the
                        # parked h. Tile 0 runs on DVE (Pool still streaming
                        # weight DMAs); tile 1 mostly Pool.
                        p, m = divmod(hc, 3)
                        eng = nc.vector if (t == 0 or m == 0) else nc.gpsimd
                        if m == 1:
                            gbs = gbmix[t][:, p, :]
                        else:
                            gbs = gb_tiles[(t, p)][:, 0 if m == 0 else 1, :]
                        hp = hpark.pop((t, hc))
                        eng.tensor_tensor(out=hg8[:, hc % 2, hc // 2, :],
                                          in0=hp[:, :], in1=gbs,
                                          op=mybir.AluOpType.mult)

                    out_ps_box = [None]

                    def head_chunk(c, ts=ts):
                        if out_ps_box[0] is None:
                            out_ps_box[0] = hps.tile([O, NT], F32, tag="h",
                                                     name="out_ps")
                        nc.tensor.matmul(out_ps_box[0][:, :], wosb[:, c, :],
                                         xsb[:, c, ts],
                                         start=(c == 0), stop=(c == DC - 1))

                    def finish_chunk(c, moe=moe, ts=ts):
                        # z' = max(moe_psum,0) + 512x in one Pool op, in place
                        nc.gpsimd.scalar_tensor_tensor(
                            out=xsb[:, c, ts], in0=moe[c][:, :], scalar=0.0,
                            in1=xsb[:, c, ts],
                            op0=mybir.AluOpType.max, op1=mybir.AluOpType.add)

                    def stage2_ktile(k, close, moe=moe, hg8=hg8, ts=ts):
                        for c in range(DC):
                            nc.tensor.matmul(moe[c][:, :],
                                             w28[:, :, k, 128 * c:128 * (c + 1)],
                                             hg8[:, :, k, :],
                                             start=(k == 0), stop=False,
                                             perf_mode=DR)
                            if close:
                                # b2 bias term closes this chunk's accumulation
                                nc.tensor.matmul(moe[c][:, :],
                                                 b28[:, :, 128 * c:128 * (c + 1)],
                                                 gTa8[:, :, ts],
                                                 start=False, stop=True,
                                                 perf_mode=DR)
                                finish_chunk(c)
                                # head matmul trails two chunks behind so its
                                # relu+residual drain is already complete
                                if c >= 2:
                                    head_chunk(c - 2)
                        if close:
                            head_chunk(DC - 2)
                            head_chunk(DC - 1)

                    # software pipeline: W1+relu run 2 k-tiles ahead; the
                    # gate-mults are emitted just before their consumer
                    for k in range(NK2):
                        hcs = (range(4, 6) if k == 0 else
                               range(2 * k + 4, min(2 * k + 6, NHC)))
                        if t == 0:
                            for hc in hcs:
                                stage1a(t, hc)
                        else:
                            if k == 0:
                                for hc in range(4):
                                    stage1a(t, hc)
                            for hc in hcs:
                                stage1a(t, hc)
                        mult_chunk(2 * k)
                        mult_chunk(2 * k + 1)
                        stage2_ktile(k, close=(k == NK2 - 1))

                    out_ps = out_ps_box[0]
                    osb = opool.tile([O, NT], F32, tag="osb")
                    for hf in range(2):
                        hs = slice(NT // 2 * hf, NT // 2 * (hf + 1))
                        nc.scalar.activation(osb[:, hs], out_ps[:, hs],
                                             mybir.ActivationFunctionType.Identity,
                                             bias=bosb[:, :])
                        nc.sync.dma_start(out=outT[:, NT * t + NT // 2 * hf:
                                                   NT * t + NT // 2 * (hf + 1)],
                                          in_=osb[:, hs])
            ctx_hpark.__exit__(None, None, None)
            ctx_hps.__exit__(None, None, None)
            ctx_gb.__exit__(None, None, None)

    nc.compile()
    return nc


def _pack_core_inputs(x, Wg, W1, b1, W2, b2, Wo, bo, c4):
    """Per-core input dict for one modality's weights + 1024-token slice."""
    f = np.float32
    f8 = ml_dtypes.float8_e4m3
    bf = ml_dtypes.bfloat16
    tok = slice(BC * c4, BC * (c4 + 1))
    x = np.asarray(x[tok], f)                                    # [BC, 768]
    xT = np.ascontiguousarray(x.T)                               # [768, BC]
    # DoubleRow pack: k = 256*kt + 128*i + p  ->  [p, i, kt, ...]
    x8 = np.ascontiguousarray(
        xT.reshape(NK1, 2, 128, BC).transpose(2, 1, 0, 3)).astype(f8)
    W1r = np.asarray(W1, f).transpose(1, 0, 2).reshape(D, E * H)  # [768, 3072]
    w18 = np.ascontiguousarray(
        (SW * W1r).reshape(NK1, 2, 128, E * H).transpose(2, 1, 0, 3)).astype(f8)
    W2r = np.asarray(W2, f).reshape(E * H, D)                     # [3072, 768]
    w28 = np.ascontiguousarray(
        (SW * W2r).reshape(NK2, 2, 128, D).transpose(2, 1, 0, 3)).astype(f8)
    # b2 DoubleRow pack: expert e = p + 8*i; scaled so (8g)@b28 = 512*(g@b2)
    b28 = np.ascontiguousarray(
        (SX / SG) * np.asarray(b2, f).reshape(2, 8, D).transpose(1, 0, 2)).astype(f8)
    return {
        "xT": np.ascontiguousarray(SX * xT).astype(bf),
        "x8d": x8,
        "w18d": w18,
        "w28d": w28,
        "b1p": np.ascontiguousarray(np.asarray(b1, f).reshape(NHC, 128).T),
        "b28d": b28,
        "wg": (np.asarray(Wg, f) / SX).astype(bf),
        "wo": (np.asarray(Wo, f) / SX).astype(bf),
        "bo": np.ascontiguousarray(np.asarray(bo, f).reshape(O, 1)),
    }


def run_on_hw(inputs, trace=False, **kw):
    if "nc" not in _NC_CACHE:
        _NC_CACHE["nc"] = build_nc()
    nc = _NC_CACHE["nc"]
    in_maps = []
    for core in range(NCORES):
        i, c4 = divmod(core, 4)
        x = inputs["x0"] if i == 0 else inputs["x1"]
        in_maps.append(_pack_core_inputs(
            x, inputs["Wg"][i], inputs["W1"][i], inputs["b1"][i],
            inputs["W2"][i], inputs["b2"][i], inputs["Wo"][i], inputs["bo"][i], c4))
    res = run_bass_kernel_spmd(nc, in_maps, core_ids=list(range(NCORES)),
                               trace=trace, **kw)
    outs = []
    for i in range(2):
        outs.append(np.concatenate(
            [res.results[4 * i + c]["outT"].T for c in range(4)], axis=0))
    return (outs[0], outs[1]), res


def kernel(**inputs):
    (o0, o1), _ = run_on_hw(inputs)
    return (o0, o1)


# revision 26
# speedup vs baseline: 2.7328x; 1.0760x over previous
"""Trainium2 Bass kernel for nn_ClassifierGuided (2-modality top-12-of-16 MoE classifier).

Sharding: pure data-parallel over tokens. 2 modalities x 4096 tokens; each of
the 8 cores owns 1024 tokens of one modality (cores 0-3 -> modality 0, cores
4-7 -> modality 1) and that modality's full weights. Dense-eval MoE (all 16
experts computed, sparse gates applied), so no all-to-all is needed.

Expert matmuls run in fp8 (e4m3) with the PE DoubleRow perf mode: each matmul
contracts 256 k-dims (2 packed rows per partition) at 0.5 cycles per output
row -- 4x the fp32r rate. Scale scheme keeps every fused op cheap:
  xsb   = 512*x   (bf16; gating/residual/head path; Wg, Wo divided by 512)
  x8    = x       (fp8 moving operand of W1)
  w18   = 64*W1, w28 = 64*W2, gates cast to fp8 as 8*g, b28 = 512*b2/8
  h_psum = 64*(x@W1) -> ACT relu(in/64 + b1) -> h parked in SBUF (fp32)
  hg8   = h * (8g) in fp8 (DVE/Pool, fused multiply + cast)
  moe_psum = hg8@(64*W2) + (8g)@(512*b2/8) = 512*(moe + b2 term)
  drain: z' = max(moe_psum,0) + xsb = 512*z (single fused op), head uses Wo/512.

Parking h in SBUF frees each h-PSUM bank right after the relu, so the
gate-broadcast DMA latency never blocks the W1 pipeline.
"""
import sys

sys.path.insert(0, "/opt/trn_rl_repo")

import ml_dtypes
import numpy as np

import concourse.bass as bass
import concourse.mybir as mybir
import concourse.tile as tile
from concourse import bacc
from concourse.bass_utils import run_bass_kernel_spmd
from concourse.masks import make_identity

# ---- problem sizes (hardcoded per the harness contract) ----
B = 4096           # tokens per modality
D = 768            # model dim
E = 16             # experts
H = 192            # expert hidden
O = 101            # classifier out
KTOP = 12          # top-k experts
NCORES = 8
BC = B // 4        # 1024 tokens per core
DC = D // 128      # 6 d-chunks
NT = 512           # token tile (matmul moving dim)
NTILES = BC // NT  # 2
NHC = E * H // 128 # 24 h-chunks of 128
NK1 = D // 256     # 3 DoubleRow k-tiles for W1 (contract d)
NK2 = E * H // 256 # 12 DoubleRow k-tiles for W2 (contract h)
NPAIR = E // 2     # 8 expert pairs (one gb broadcast tile each)
F32 = mybir.dt.float32
BF16 = mybir.dt.bfloat16
F8 = mybir.dt.float8e4
DR = mybir.MatmulPerfMode.DoubleRow
NEG_BIG = -1.0e30

SX = 512.0   # x residual scale (Wg, Wo pre-divided on host)
SW = 64.0    # fp8 weight scale for W1/W2
SG = 8.0     # fp8 gate scale  (SW * SG == SX keeps the drain exact)

_NC_CACHE = {}


def build_nc():
    nc = bacc.Bacc("TRN2", target_bir_lowering=False, debug=False,
                   num_devices=NCORES)

    # ---- DRAM I/O (per-core views; host pre-packs) ----
    xT = nc.dram_tensor("xT", [D, BC], BF16, kind="ExternalInput").ap()       # 512*x
    x8d = nc.dram_tensor("x8d", [128, 2, NK1, BC], F8, kind="ExternalInput").ap()
    w18d = nc.dram_tensor("w18d", [128, 2, NK1, E * H], F8, kind="ExternalInput").ap()
    w28d = nc.dram_tensor("w28d", [128, 2, NK2, D], F8, kind="ExternalInput").ap()
    b1p = nc.dram_tensor("b1p", [128, NHC], F32, kind="ExternalInput").ap()
    b28d = nc.dram_tensor("b28d", [8, 2, D], F8, kind="ExternalInput").ap()
    wg = nc.dram_tensor("wg", [D, E], BF16, kind="ExternalInput").ap()        # Wg/512
    wo = nc.dram_tensor("wo", [D, O], BF16, kind="ExternalInput").ap()        # Wo/512
    bo = nc.dram_tensor("bo", [1, O], BF16, kind="ExternalInput").ap()
    outT = nc.dram_tensor("outT", [O, BC], F32, kind="ExternalOutput").ap()

    with tile.TileContext(nc) as tc:
        with tc.tile_pool(name="const", bufs=1) as cpool:
            # resident SBUF tensors
            xsb = cpool.tile([128, DC, BC], BF16)            # 512*xT; later 512*zT
            x8 = cpool.tile([128, 2, NK1, BC], F8)
            w18 = cpool.tile([128, 2, NK1, E * H], F8)
            w28 = cpool.tile([128, 2, NK2, D], F8)
            b1sb = cpool.tile([128, NHC], F32)
            b28 = cpool.tile([8, 2, D], F8)
            wgf = cpool.tile([128, DC, E], BF16)
            wosb = cpool.tile([128, DC, O], BF16)
            bosb = cpool.tile([1, O], BF16)
            ones = cpool.tile([1, NT], BF16)
            nc.gpsimd.memset(ones[:, :], 1.0)
            gT = cpool.tile([E, BC], F32)                    # gates, expert-major
            gT8 = cpool.tile([E, BC], F8)                    # 8*gates, fp8
            gTa8 = cpool.tile([8, 2, BC], F8)                # 8*gates, DoubleRow view
            ident = cpool.tile([128, 128], F32)

            make_identity(nc, ident[:, :])

            # ---- loads, per-queue FIFO in need order ----
            # SP: gating weights + half of x (gating-critical), then the gate
            # flush/broadcast DMAs (emitted inline) and the output writes.
            # ACT: other half of x (ACT compute starts only at first relu).
            # Pool: fp8 x + expert weights, interleaved in use order.
            nc.sync.dma_start(out=wgf[:, :, :],
                              in_=wg.rearrange("(c p) e -> p c e", p=128))
            for c in (0, 2, 4):
                nc.sync.dma_start(out=xsb[:, c, :], in_=xT[128 * c:128 * (c + 1), :])
            nc.sync.dma_start(out=b1sb[:, :], in_=b1p[:, :])
            for c in (1, 3, 5):
                nc.scalar.dma_start(out=xsb[:, c, :], in_=xT[128 * c:128 * (c + 1), :])
            # Pool, in use order: x8 + first W1 columns, then W2 k-tiles
            # interleaved with later W1 columns so both streams stay ahead.
            for kt in range(NK1):
                nc.gpsimd.dma_start(out=x8[:, :, kt, :], in_=x8d[:, :, kt, :])
            HQ = E * H // 4
            for kt in range(NK1):
                nc.gpsimd.dma_start(out=w18[:, :, kt, 0:HQ],
                                    in_=w18d[:, :, kt, 0:HQ])
            for g2 in range(3):
                nc.gpsimd.dma_start(out=w28[:, :, 2 * g2:2 * g2 + 2, :],
                                    in_=w28d[:, :, 2 * g2:2 * g2 + 2, :])
                for kt in range(NK1):
                    nc.gpsimd.dma_start(
                        out=w18[:, :, kt, HQ * (g2 + 1):HQ * (g2 + 2)],
                        in_=w18d[:, :, kt, HQ * (g2 + 1):HQ * (g2 + 2)])
            for g2 in range(3, 6):
                nc.gpsimd.dma_start(out=w28[:, :, 2 * g2:2 * g2 + 2, :],
                                    in_=w28d[:, :, 2 * g2:2 * g2 + 2, :])
            nc.gpsimd.dma_start(out=b28[:, :, :], in_=b28d)
            nc.gpsimd.dma_start(out=wosb[:, :, :],
                                in_=wo.rearrange("(c p) o -> p c o", p=128))
            nc.gpsimd.dma_start(out=bosb[:, :], in_=bo[:, :])

            # gates round-trip through DRAM (fp8); gate-broadcast tiles are
            # read back with partition-step-0 DMAs
            gdram8 = cpool.tile([E, BC], F8, space="DRAM")
            gb_tiles = {}
            gbmix = {}
            ctx_gb = tc.tile_pool(name="gbpool", bufs=2 * NPAIR + 2)
            gbpool = ctx_gb.__enter__()

            def load_gb(t, p):
                # gb[:,0,:] = 8*gate(e0) broadcast over partitions; [:,1,:] = e1
                gb = gbpool.tile([128, 2, NT], F8, tag="gb", name="gb")
                gb_src = bass.AP(tensor=gdram8.tensor,
                                 offset=2 * p * BC + NT * t,
                                 ap=[[0, 128], [BC, 2], [1, NT]])
                nc.sync.dma_start(out=gb[:, :, :], in_=gb_src)
                gb_tiles[(t, p)] = gb

            def load_gbmix_group(t, q):
                # mixed-chunk gate tile, pair group q (pairs 2q, 2q+1): for
                # pair p, partitions 0:64 carry e(2p) and 64:128 carry
                # e(2p+1) -- matches the partition split of the middle
                # h-chunk, so its gate-mult is one op
                gm = gbmix[t]
                for half in range(2):
                    src = bass.AP(tensor=gdram8.tensor,
                                  offset=half * BC + 4 * q * BC + NT * t,
                                  ap=[[0, 64], [2 * BC, 2], [1, NT]])
                    nc.sync.dma_start(
                        out=gm[64 * half:64 * (half + 1), 2 * q:2 * q + 2, :],
                        in_=src)

            def flush_gates(t):
                ts = slice(NT * t, NT * (t + 1))
                # cast fp32 gates -> 8*g in fp8, then flush for broadcast reads
                nc.vector.tensor_scalar_mul(gT8[:, ts], gT[:, ts], SG)
                nc.sync.dma_start(out=gdram8[:, ts], in_=gT8[:, ts])
                # DoubleRow-packed gate view for the b2 bias matmul
                ga_src = bass.AP(tensor=gdram8.tensor, offset=NT * t,
                                 ap=[[BC, 8], [8 * BC, 2], [1, NT]])
                nc.sync.dma_start(out=gTa8[:, :, ts], in_=ga_src)
                gbmix[t] = gbpool.tile([128, NPAIR, NT], F8, tag="gbm",
                                       name="gbm")
                # interleave pure-pair and mixed-group loads in consumer order
                load_gb(t, 0)
                load_gbmix_group(t, 0)
                load_gb(t, 1)
                for q in range(1, NPAIR // 2):
                    load_gb(t, 2 * q)
                    load_gbmix_group(t, q)
                    load_gb(t, 2 * q + 1)

            # ---------------- main-loop machinery ----------------
            ctx_hps = tc.tile_pool(name="hps", bufs=2, space="PSUM")
            hps = ctx_hps.__enter__()
            ctx_hpark = tc.tile_pool(name="hpark", bufs=10)
            hpark_pool = ctx_hpark.__enter__()
            hpark = {}

            def stage1a(t, hc):
                # W1 (fp8 DoubleRow) + relu; h parked in SBUF, PSUM bank freed
                ts = slice(NT * t, NT * (t + 1))
                hps_t = hps.tile([128, NT], F32, tag="h")
                for kt in range(NK1):
                    nc.tensor.matmul(hps_t[:, :],
                                     w18[:, :, kt, 128 * hc:128 * (hc + 1)],
                                     x8[:, :, kt, ts],
                                     start=(kt == 0), stop=(kt == NK1 - 1),
                                     perf_mode=DR)
                hp = hpark_pool.tile([128, NT], F32, tag="hp", name="hp")
                nc.scalar.activation(hp[:, :], hps_t[:, :],
                                     mybir.ActivationFunctionType.Relu,
                                     bias=b1sb[:, hc:hc + 1], scale=1.0 / SW)
                hpark[(t, hc)] = hp

            # ---------------- gating pass (128-token subtiles) ----------------
            # W1 for the first few h-chunks is emitted first: it only needs
            # x8/w18, so the PE warms up while the bf16 x chunks still stream.
            for hc in range(6):
                stage1a(0, hc)

            with tc.tile_pool(name="gps", bufs=2, space="PSUM") as gps, \
                 tc.tile_pool(name="gtp", bufs=2, space="PSUM") as gtp, \
                 tc.tile_pool(name="gsb", bufs=3) as gsb:
                for i in range(BC // 128):
                    if i * 128 % NT == 0 and i > 0:
                        flush_gates(i * 128 // NT - 1)
                    ts = slice(128 * i, 128 * (i + 1))
                    lg_ps = gps.tile([128, E], F32, tag="lg")
                    for c in range(DC):
                        nc.tensor.matmul(lg_ps[:, :], xsb[:, c, ts],
                                         wgf[:, c, :],
                                         start=(c == 0), stop=(c == DC - 1))
                    lg = gsb.tile([128, E], F32, tag="lg_sb")
                    nc.vector.tensor_copy(lg[:, :], lg_ps[:, :])
                    # top-8 values, then values 9..16 after masking them out
                    t8a = gsb.tile([128, 8], F32, tag="t8a")
                    nc.vector.max(t8a[:, :], lg[:, :])
                    l2 = gsb.tile([128, E], F32, tag="l2")
                    nc.vector.match_replace(l2[:, :], t8a[:, :], lg[:, :], NEG_BIG)
                    t8b = gsb.tile([128, 8], F32, tag="t8b")
                    nc.vector.max(t8b[:, :], l2[:, :])
                    # softmax over entries >= 12th-largest (t8b[:,3])
                    e16 = gsb.tile([128, E], F32, tag="e16")
                    nc.scalar.activation(e16[:, :], lg[:, :],
                                         mybir.ActivationFunctionType.Exp)
                    em = gsb.tile([128, E], F32, tag="em")
                    ssum = gsb.tile([128, 1], F32, tag="ssum")
                    nc.vector.scalar_tensor_tensor(
                        out=em[:, :], in0=lg[:, :], scalar=t8b[:, 3:4],
                        in1=e16[:, :], op0=mybir.AluOpType.is_ge,
                        op1=mybir.AluOpType.mult, accum_out=ssum[:, :])
                    rinv = gsb.tile([128, 1], F32, tag="rinv")
                    nc.vector.reciprocal(rinv[:, :], ssum[:, :])
                    g = gsb.tile([128, E], F32, tag="g")
                    nc.vector.tensor_scalar_mul(g[:, :], em[:, :], rinv[:, :])
                    # transpose to expert-major gT[16, tokens]
                    gt_ps = gtp.tile([E, 128], F32, tag="gt")
                    nc.tensor.transpose(gt_ps[:, :], g[:, :], ident[:, :])
                    nc.vector.tensor_copy(gT[:, ts], gt_ps[:, :])

            flush_gates(NTILES - 1)

            # ---------------- main loop ----------------
            with tc.tile_pool(name="moeps", bufs=DC, space="PSUM") as moeps, \
                 tc.tile_pool(name="hgpool", bufs=2) as hgpool, \
                 tc.tile_pool(name="opool", bufs=4) as opool:
                for t in range(NTILES):
                    ts = slice(NT * t, NT * (t + 1))
                    moe = [moeps.tile([128, NT], F32, tag="moe", name="moe")
                           for _ in range(DC)]
                    hg8 = hgpool.tile([128, 2, NK2, NT], F8, tag="hg", name="hg")

                    def mult_chunk(hc, t=t, hg8=hg8):
                        # hg8 = h * (8g): fused multiply + fp8 cast from the
                        # parked h. Tile 0 runs on DVE (Pool still streaming
                        # weight DMAs); tile 1 mostly Pool.
                        p, m = divmod(hc, 3)
                        eng = nc.vector if (t == 0 or m == 0) else nc.gpsimd
                        if m == 1:
                            gbs = gbmix[t][:, p, :]
                        else:
                            gbs = gb_tiles[(t, p)][:, 0 if m == 0 else 1, :]
                        hp = hpark.pop((t, hc))
                        eng.tensor_tensor(out=hg8[:, hc % 2, hc // 2, :],
                                          in0=hp[:, :], in1=gbs,
                                          op=mybir.AluOpType.mult)

                    # head runs as two token-halves (384/128) in PSUM tiles
                    # reclaimed from the moe pool; bo is folded in as a
                    # ones-row matmul and outT DMAs read PSUM directly, so
                    # the tail is just the short half's close + DMA
                    HSPL = (slice(0, 384), slice(384, NT))
                    head_ps = [None, None]

                    def head_chunk(c, t=t):
                        for hf in range(2):
                            if head_ps[hf] is None:
                                head_ps[hf] = moeps.tile(
                                    [O, HSPL[hf].stop - HSPL[hf].start], F32,
                                    tag="moe", name="head")
                            nc.tensor.matmul(
                                head_ps[hf][:, :], wosb[:, c, :],
                                xsb[:, c, NT * t + HSPL[hf].start:
                                    NT * t + HSPL[hf].stop],
                                start=(c == 0), stop=False)

                    def head_close(t=t):
                        for hf in range(2):
                            hs = HSPL[hf]
                            nc.tensor.matmul(head_ps[hf][:, :], bosb[:, :],
                                             ones[:, hs], start=False,
                                             stop=True)
                            osb = opool.tile([O, hs.stop - hs.start], F32,
                                             tag="osb", name="osb")
                            nc.vector.tensor_copy(osb[:, :], head_ps[hf][:, :])
                            nc.sync.dma_start(
                                out=outT[:, NT * t + hs.start:NT * t + hs.stop],
                                in_=osb[:, :])

                    def finish_chunk(c, moe=moe, ts=ts):
                        # z' = max(moe_psum,0) + 512x in one Pool op, in place
                        nc.gpsimd.scalar_tensor_tensor(
                            out=xsb[:, c, ts], in0=moe[c][:, :], scalar=0.0,
                            in1=xsb[:, c, ts],
                            op0=mybir.AluOpType.max, op1=mybir.AluOpType.add)

                    def stage2_ktile(k, close, moe=moe, hg8=hg8, ts=ts):
                        for c in range(DC):
                            nc.tensor.matmul(moe[c][:, :],
                                             w28[:, :, k, 128 * c:128 * (c + 1)],
                                             hg8[:, :, k, :],
                                             start=(k == 0), stop=False,
                                             perf_mode=DR)
                            if close:
                                # b2 bias term closes this chunk's accumulation
                                nc.tensor.matmul(moe[c][:, :],
                                                 b28[:, :, 128 * c:128 * (c + 1)],
                                                 gTa8[:, :, ts],
                                                 start=False, stop=True,
                                                 perf_mode=DR)
                                finish_chunk(c)
                                # head matmul trails two chunks behind so its
                                # relu+residual drain is already complete
                                if c >= 2:
                                    head_chunk(c - 2)
                        if close:
                            head_chunk(DC - 2)
                            head_chunk(DC - 1)
                            head_close()

                    # software pipeline: W1+relu run 2-3 k-tiles ahead; the
                    # gate-mults are emitted just before their consumer
                    for k in range(NK2):
                        if t == 1 and k == 0:
                            for hc in range(6):
                                stage1a(t, hc)
                        for hc in range(2 * k + 6, min(2 * k + 8, NHC)):
                            stage1a(t, hc)
                        mult_chunk(2 * k)
                        mult_chunk(2 * k + 1)
                        stage2_ktile(k, close=(k == NK2 - 1))
            ctx_hpark.__exit__(None, None, None)
            ctx_hps.__exit__(None, None, None)
            ctx_gb.__exit__(None, None, None)

    nc.compile()
    return nc


def _pack_core_inputs(x, Wg, W1, b1, W2, b2, Wo, bo, c4):
    """Per-core input dict for one modality's weights + 1024-token slice."""
    f = np.float32
    f8 = ml_dtypes.float8_e4m3
    bf = ml_dtypes.bfloat16
    tok = slice(BC * c4, BC * (c4 + 1))
    x = np.asarray(x[tok], f)                                    # [BC, 768]
    xT = np.ascontiguousarray(x.T)                               # [768, BC]
    # DoubleRow pack: k = 256*kt + 128*i + p  ->  [p, i, kt, ...]
    x8 = np.ascontiguousarray(
        xT.reshape(NK1, 2, 128, BC).transpose(2, 1, 0, 3)).astype(f8)
    W1r = np.asarray(W1, f).transpose(1, 0, 2).reshape(D, E * H)  # [768, 3072]
    w18 = np.ascontiguousarray(
        (SW * W1r).reshape(NK1, 2, 128, E * H).transpose(2, 1, 0, 3)).astype(f8)
    W2r = np.asarray(W2, f).reshape(E * H, D)                     # [3072, 768]
    w28 = np.ascontiguousarray(
        (SW * W2r).reshape(NK2, 2, 128, D).transpose(2, 1, 0, 3)).astype(f8)
    # b2 DoubleRow pack: expert e = p + 8*i; scaled so (8g)@b28 = 512*(g@b2)
    b28 = np.ascontiguousarray(
        (SX / SG) * np.asarray(b2, f).reshape(2, 8, D).transpose(1, 0, 2)).astype(f8)
    return {
        "xT": np.ascontiguousarray(SX * xT).astype(bf),
        "x8d": x8,
        "w18d": w18,
        "w28d": w28,
        "b1p": np.ascontiguousarray(np.asarray(b1, f).reshape(NHC, 128).T),
        "b28d": b28,
        "wg": (np.asarray(Wg, f) / SX).astype(bf),
        "wo": (np.asarray(Wo, f) / SX).astype(bf),
        "bo": np.ascontiguousarray(np.asarray(bo, f).reshape(1, O)).astype(bf),
    }


def run_on_hw(inputs, trace=False, **kw):
    if "nc" not in _NC_CACHE:
        _NC_CACHE["nc"] = build_nc()
    nc = _NC_CACHE["nc"]
    in_maps = []
    for core in range(NCORES):
        i, c4 = divmod(core, 4)
        x = inputs["x0"] if i == 0 else inputs["x1"]
        in_maps.append(_pack_core_inputs(
            x, inputs["Wg"][i], inputs["W1"][i], inputs["b1"][i],
            inputs["W2"][i], inputs["b2"][i], inputs["Wo"][i], inputs["bo"][i], c4))
    res = run_bass_kernel_spmd(nc, in_maps, core_ids=list(range(NCORES)),
                               trace=trace, **kw)
    outs = []
    for i in range(2):
        outs.append(np.concatenate(
            [res.results[4 * i + c]["outT"].T for c in range(4)], axis=0))
    return (outs[0], outs[1]), res


def kernel(**inputs):
    (o0, o1), _ = run_on_hw(inputs)
    return (o0, o1)
